# revision 20
# baseline (speedup 1.0000x reference)
"""Trainium2 Bass kernel for nn_EquivariantTransformerBlock.

Strategy (8 NeuronCores, no collectives, no indirect DMA):
  - Host assigns each node to one of 320 buckets of 128 nodes (degree-
    balanced snake packing); every edge goes to the core owning its dst
    bucket, so all segment sums are core-local.
  - Host computes the (tiny) equivariant LayerNorm, gathers f[src], and
    precomputes per-chunk one-hot matrices (fp8) for the segment sums.
  - Device pipeline, bucket (KB x 128 edges) at a time, software-
    pipelined one bucket ahead (PE MLP + ScalarE casts + DMA run under
    the DVE cascade of the previous bucket):
      * edge MLP on the TensorE in fp16; PSUM->SBUF casts on ScalarE,
      * fU x basis and rw x tmp contractions as fp16 broadcast-product
        + halving-tree ops on the VectorE (fp16 keeps the DVE 2x mode
        with 4x finer rounding than bf16; scores need that precision),
      * attention scores with the 1/sqrt(h) scale folded into the
        k-rows of w2 on the host,
      * dual-exp softmax (shift-free: clamped exp(s) / exp(s-140) with
        per-node select in the epilogue) with bf16 payload range,
      * segment sums as one-hot matmuls accumulated in PSUM per bucket.
"""

import math
from contextlib import ExitStack
from dataclasses import dataclass

import numpy as np
import ml_dtypes

BF16 = ml_dtypes.bfloat16

N_NODES = 40000
N_EDGES = 320000
M1, D1 = 8, 4
M2, D2 = 8, 4
LN_EPS = 1e-5
EQ_EPS = 1e-8
IX1 = np.array([0, 1, 1, 1])
IX2 = np.array([0, 1, 1, 1])

N_CORES = 8
BUCKET_N = 128
NB = 40
SCALE = 32.0 ** -0.5
SHIFT_B = 140.0
CLAMP_A = 1e34
SEL_TH = 1e33

# Precision knobs (validated against numpy sim of the same pipeline):
#   mlp_f32: run the edge MLP matmuls in fp32
#   tmp_f32: run the fU x basis cascade in fp32 (quantize tmp2 to bf16)
OPT = {"mlp_f32": False, "tmp_f32": False, "tmp_gpsimd": False, "tmp_5d": False, "oh_fp8": True}


@dataclass
class Cfg:
    nb: int
    kb: int

    @property
    def ch(self):
        return self.nb * self.kb

    @property
    def e_pad(self):
        return self.ch * 128


# ---------------------------------------------------------------------------
# Patches: this walrus build allows at most ONE sync wait per instruction.
# ---------------------------------------------------------------------------
_PATCHED = False


def _apply_patches():
    global _PATCHED
    if _PATCHED:
        return
    _PATCHED = True
    import re as _re

    import orjson as _orjson

    import concourse.bass as _bass
    from concourse.tile import TileContext as _TC
    from concourse.vector_clock import ScopedClock as _SC, VectorClock as _VC

    def _drain_and_barrier(self, tick_clock, wait_clock):
        nc = self.nc
        gvals = [int(x) for x in _re.findall(r"\d+", repr(tick_clock.global_clock))]
        nz = [(p, v) for p, v in enumerate(gvals) if v > 0]
        if not nz:
            nc.sync.drain()
        for p, v in nz:
            pvc = _VC()
            pvc.require_at_least(p, v)
            d = nc.sync.drain()
            wait_clock.add_sem_waits(d.ins, _SC({None: pvc}))
        nc.all_engine_barrier()
        assert self.sems is not None
        popped = nc._tile_sem_poison_stack.pop()
        assert popped is self._sem_poison
        nc.clear_and_free_semaphores(list(self.sems.allocated().values()))
        nc.all_engine_barrier()

    def _split_multi_waits(data: bytes) -> bytes:
        j = _orjson.loads(data)
        for fn in j.get("functions", []):
            for bb in fn.get("blocks", []):
                out = []
                for ins in bb.get("instructions", []):
                    si = ins.get("sync_info")
                    ow = (si or {}).get("on_wait") or []
                    if len(ow) > 1:
                        for k, w in enumerate(ow[:-1]):
                            out.append({
                                "debug": ins.get("debug", 0),
                                "engine": ins["engine"],
                                "ins": [],
                                "name": f"{ins['name']}-spw{k}",
                                "opcode": "EventSemaphore",
                                "outs": [],
                                "sync_info": {"on_update": [], "on_wait": [w]},
                            })
                        si["on_wait"] = [ow[-1]]
                    out.append(ins)
                bb["instructions"] = out
        return _orjson.dumps(j)

    _orig_to_json_bytes = _bass.Bass.to_json_bytes

    def _to_json_bytes(self):
        return _split_multi_waits(_orig_to_json_bytes(self))

    _TC._drain_and_barrier = _drain_and_barrier
    _bass.Bass.to_json_bytes = _to_json_bytes


# ---------------------------------------------------------------------------
# Device kernel
# ---------------------------------------------------------------------------
def build_kernel(nc, cfg: Cfg):
    import concourse.bass as bass
    import concourse.mybir as mybir
    from concourse.tile import TileContext

    f32 = mybir.dt.float32
    bf16 = mybir.dt.bfloat16
    fp16 = mybir.dt.float16
    Alu = mybir.AluOpType
    Act = mybir.ActivationFunctionType

    NBk, KB, CH, E_PAD = cfg.nb, cfg.kb, cfg.ch, cfg.e_pad
    HB = KB // 2            # half-bucket chunk count for the big cascade
    mdt = f32 if OPT["mlp_f32"] else fp16
    tdt = f32 if OPT["tmp_f32"] else fp16
    cdt = fp16

    basis_d = nc.dram_tensor("basis_s", (NBk, 128, KB * 64), tdt,
                             kind="ExternalInput")
    fu_d = nc.dram_tensor("fu_s", (NBk, 128, KB * 32), tdt,
                          kind="ExternalInput")
    eft_d = nc.dram_tensor("eft_s", (32, E_PAD), mdt, kind="ExternalInput")
    fp8 = mybir.dt.float8e4
    ohdt = fp8 if OPT.get("oh_fp8") else bf16
    oh_d = nc.dram_tensor("oh_s", (NBk, 128, KB * 128), ohdt,
                          kind="ExternalInput")
    w1t_d = nc.dram_tensor("w1t_s", (32, 64), mdt, kind="ExternalInput")
    b1_d = nc.dram_tensor("b1_s", (64, 1), f32, kind="ExternalInput")
    w2b_d = nc.dram_tensor("w2b_s", (65, 768), mdt, kind="ExternalInput")
    proj_d = nc.dram_tensor("proj_s", (128, 256), bf16, kind="ExternalInput")
    out_d = nc.dram_tensor("out_s", (NBk * 128, 32), bf16,
                           kind="ExternalOutput")

    def vap(base, offset, dims):
        return bass.AP(base.tensor, base.offset + offset, dims)

    with TileContext(nc) as tc:
        with ExitStack() as ctx:
            cpool = ctx.enter_context(tc.tile_pool(name="consts", bufs=1))
            w1t_t = cpool.tile([32, 64], mdt)
            nc.sync.dma_start(out=w1t_t[:], in_=w1t_d.ap())
            b1_t = cpool.tile([64, 1], f32)
            nc.sync.dma_start(out=b1_t[:], in_=b1_d.ap())
            w2b_t = cpool.tile([65, 768], mdt)
            nc.sync.dma_start(out=w2b_t[:], in_=w2b_d.ap())
            proj_t = cpool.tile([128, 256], bf16)
            nc.sync.dma_start(out=proj_t[:], in_=proj_d.ap())
            segS = cpool.tile([128, NBk * 72], f32)
            shiftB = cpool.tile([128, 1], f32)
            nc.vector.memset(shiftB[:], -SHIFT_B)

            bpool = ctx.enter_context(tc.tile_pool(name="edges", bufs=2))
            rpool = ctx.enter_context(tc.tile_pool(name="rw", bufs=2))
            gpool = ctx.enter_context(tc.tile_pool(name="tmp", bufs=2))
            tpool = ctx.enter_context(tc.tile_pool(name="work", bufs=1))
            hpool = ctx.enter_context(
                tc.tile_pool(name="psH", bufs=2, space="PSUM"))
            ppool = ctx.enter_context(
                tc.tile_pool(name="psR", bufs=2, space="PSUM"))
            spool = ctx.enter_context(
                tc.tile_pool(name="psS", bufs=2, space="PSUM"))

            # persistent h65 pair with the ones-row set once
            h65s = [cpool.tile([65, 128], mdt, name=f"h65p{k}")
                    for k in range(2)]
            for t in h65s:
                nc.gpsimd.memset(t[64:65, :], 1.0)

            KB2 = 2 * KB

            def load_pair(k):
                """DMA both buckets of pair k into paired tiles."""
                b0 = 2 * k
                basis2 = bpool.tile([128, KB2 * 64], tdt, tag="basisb")
                nc.sync.dma_start(
                    out=basis2[:],
                    in_=vap(basis_d.ap(), b0 * 128 * KB * 64,
                            [[KB * 64, 128], [128 * KB * 64, 2],
                             [1, KB * 64]]))
                fu2 = bpool.tile([128, KB2 * 32], tdt, tag="fub")
                nc.sync.dma_start(
                    out=fu2[:],
                    in_=vap(fu_d.ap(), b0 * 128 * KB * 32,
                            [[KB * 32, 128], [128 * KB * 32, 2],
                             [1, KB * 32]]))
                eft2 = bpool.tile([32, KB2 * 128], mdt, tag="eftb")
                nc.sync.dma_start(
                    out=eft2[:],
                    in_=vap(eft_d.ap(), b0 * KB * 128,
                            [[E_PAD, 32], [1, KB2 * 128]]))
                oh2 = bpool.tile([128, KB2 * 128], ohdt, tag="ohb")
                nc.sync.dma_start(
                    out=oh2[:],
                    in_=vap(oh_d.ap(), b0 * 128 * KB * 128,
                            [[KB * 128, 128], [128 * KB * 128, 2],
                             [1, KB * 128]]))
                return basis2, fu2, eft2, oh2

            def mlp_bucket(b, eft2, coff):
                rwb = rpool.tile([128, KB * 768], cdt, tag="rwb")
                for i in range(KB):
                    hps = hpool.tile([64, 128], f32, tag="hps")
                    nc.tensor.matmul(out=hps[:], lhsT=w1t_t[:],
                                     rhs=eft2[:, coff + i * 128:
                                              coff + (i + 1) * 128],
                                     start=True, stop=True)
                    h65 = h65s[i % 2]
                    nc.scalar.activation(h65[0:64, :], hps[:], Act.Relu,
                                         bias=b1_t[:, 0:1])
                    rwp = ppool.tile([128, 768], f32, tag="rwp")
                    nc.tensor.matmul(out=rwp[:, 0:512], lhsT=h65[:],
                                     rhs=w2b_t[:, 0:512], start=True,
                                     stop=True)
                    nc.tensor.matmul(out=rwp[:, 512:768], lhsT=h65[:],
                                     rhs=w2b_t[:, 512:768], start=True,
                                     stop=True)
                    nc.scalar.activation(rwb[:, i * 768:(i + 1) * 768],
                                         rwp[:], Act.Copy)
                return rwb

            def tmp_pair(basis2, fu2):
                """fU x basis products + d1-sum for a bucket PAIR.

                The d1-sum is a 2-level halving tree instead of a
                tensor_reduce: reduce only has a 1x uop (FD cycles), while
                the first tree level runs at 2x (pairs (d1,d1+2) keep both
                operands stride-1 / 4B-aligned).

                ptb layout per edge: [pd(16):32][m(8):4][d1(4):1]; the host
                permutes basis cols to pd=(d2,p) and w2 rows to j=(p,m), so
                the tree's natural group order 8*pd+m lands tmp2 directly
                in P0's [d2:32][(p,m):1] layout -- every tree op then has a
                unit-stride output (strided DVE writes cost ~4.3 cyc/elem).
                """
                tmp2 = gpool.tile([128, KB2 * 128], cdt, tag="tmp2")
                ptba = ptb2[:]
                h1a = h12[:]
                nc.vector.tensor_tensor(
                    vap(ptba, 0,
                        [[KB2 * 512, 128], [512, KB2], [32, 16], [4, 8],
                         [1, 4]]),
                    vap(fu2[:], 0,
                        [[KB2 * 32, 128], [32, KB2], [0, 16], [4, 8],
                         [1, 4]]),
                    vap(basis2[:], 0,
                        [[KB2 * 64, 128], [64, KB2], [4, 16], [0, 8],
                         [1, 4]]),
                    Alu.mult)
                with nc.allow_low_precision(reason="fp16 tmp"):
                    nc.vector.tensor_tensor(
                        vap(h1a, 0,
                            [[KB2 * 256, 128], [256, KB2], [2, 128],
                             [1, 2]]),
                        vap(ptba, 0,
                            [[KB2 * 512, 128], [512, KB2], [4, 128],
                             [1, 2]]),
                        vap(ptba, 2,
                            [[KB2 * 512, 128], [512, KB2], [4, 128],
                             [1, 2]]),
                        Alu.add)
                    nc.vector.tensor_tensor(
                        vap(tmp2[:], 0,
                            [[KB2 * 128, 128], [128, KB2], [1, 128]]),
                        vap(h1a, 0,
                            [[KB2 * 256, 128], [256, KB2], [2, 128]]),
                        vap(h1a, 1,
                            [[KB2 * 256, 128], [256, KB2], [2, 128]]),
                        Alu.add)
                return tmp2

            # Pair-level tiles: the cascade fills half-slots per bucket and
            # the score/payload ops run once per 2 buckets with doubled FD,
            # halving their fixed per-instruction cost (~151 DVE cycles).
            ptb2 = tpool.tile([128, KB2 * 512], tdt, name="ptb2")
            h12 = tpool.tile([128, KB2 * 256], cdt, name="h12")
            conv2 = tpool.tile([128, KB2 * 64], cdt, name="conv2")
            convV2 = tpool.tile([128, KB2 * 32], bf16, name="convV2")
            Y2 = tpool.tile([128, KB2 * 72], bf16, name="Y2")
            ex82 = tpool.tile([128, KB2 * 64], bf16, name="ex82")
            ps2 = tpool.tile([128, KB2 * 32], cdt, name="ps2")
            sc42 = tpool.tile([128, KB2 * 4], f32, name="sc42")
            scl2 = tpool.tile([128, KB2 * 4], f32, name="scl2")

            def conv_bucket(b, rwb, tmp2):
                rwba = rwb[:]
                tmp2a = tmp2[:]
                half = (b & 1) * KB
                P0 = tpool.tile([128, KB * 3072], cdt, tag="P0")
                T1 = tpool.tile([128, KB * 1536], cdt, tag="T1")
                T2 = tpool.tile([128, KB * 768], cdt, tag="T2")
                nc.vector.tensor_tensor(
                    vap(P0[:], 0,
                        [[KB * 3072, 128], [3072, KB], [128, 24],
                         [32, 4], [1, 32]]),
                    vap(rwba, 0,
                        [[KB * 768, 128], [768, KB], [32, 24], [0, 4],
                         [1, 32]]),
                    vap(tmp2a, half * 128,
                        [[KB2 * 128, 128], [128, KB], [0, 24], [32, 4],
                         [1, 32]]),
                    Alu.mult)
                # Halving tree over j; T3/T4 live inside T1's buffer (T1 is
                # dead once T2 is written) to save 9KB/lane of SBUF.
                plan = [
                    (T1[:], 0, KB * 1536, 16),
                    (T2[:], 0, KB * 768, 8),
                    (T1[:], 0, KB * 1536, 4),
                    (T1[:], KB * 768, KB * 1536, 2),
                ]
                cur, coff, cpitch, w = P0[:], 0, KB * 3072, 32
                for nxt, noff, npitch, w2 in plan:
                    nc.vector.tensor_tensor(
                        vap(nxt, noff,
                            [[npitch, 128], [96 * w2, KB],
                             [4 * w2, 24], [w2, 4], [1, w2]]),
                        vap(cur, coff,
                            [[cpitch, 128], [96 * w, KB],
                             [4 * w, 24], [w, 4], [1, w2]]),
                        vap(cur, coff + w2,
                            [[cpitch, 128], [96 * w, KB],
                             [4 * w, 24], [w, 4], [1, w2]]),
                        Alu.add)
                    cur, coff, cpitch, w = nxt, noff, npitch, w2
                nc.vector.tensor_tensor(
                    vap(conv2[:], half * 64,
                        [[KB2 * 64, 128], [64, KB], [4, 16], [1, 4]]),
                    vap(cur, coff,
                        [[cpitch, 128], [192, KB], [8, 16], [2, 4]]),
                    vap(cur, coff + 1,
                        [[cpitch, 128], [192, KB], [8, 16], [2, 4]]),
                    Alu.add)
                nc.vector.tensor_tensor(
                    vap(convV2[:], half * 32,
                        [[KB2 * 32, 128], [32, KB], [4, 8], [1, 4]]),
                    vap(cur, coff + 128,
                        [[cpitch, 128], [192, KB], [8, 8], [2, 4]]),
                    vap(cur, coff + 129,
                        [[cpitch, 128], [192, KB], [8, 8], [2, 4]]),
                    Alu.add)

            def scores_pair(b0, oh2):
                """Scores + dual-exp softmax + payload for buckets b0,b0+1."""
                Ya = Y2[:]
                nc.vector.tensor_tensor(
                    ps2[:],
                    vap(conv2[:], 0,
                        [[KB2 * 64, 128], [64, KB2], [1, 32]]),
                    vap(conv2[:], 32,
                        [[KB2 * 64, 128], [64, KB2], [1, 32]]),
                    Alu.mult)
                nc.vector.tensor_reduce(
                    sc42[:],
                    vap(ps2[:], 0,
                        [[KB2 * 32, 128], [8, KB2 * 4], [1, 8]]),
                    mybir.AxisListType.X, Alu.add)
                # SCALE is folded into the k-rows of w2 on the host.
                # (ScalarE Lrelu was tried here: it thrashes the activation
                # table -- 1.3us ACT_TABLE_LOAD per swap vs Exp -- and its
                # alpha lowering looked wrong on HW. Keep leaky on DVE.)
                nc.vector.scalar_tensor_tensor(
                    scl2[:], sc42[:], 0.2, sc42[:], Alu.mult, Alu.max)
                nc.scalar.activation(
                    vap(Ya, 32, [[KB2 * 72, 128], [72, KB2], [1, 4]]),
                    scl2[:], Act.Exp)
                nc.vector.tensor_scalar(
                    vap(Ya, 32, [[KB2 * 72, 128], [72, KB2], [1, 4]]),
                    vap(Ya, 32, [[KB2 * 72, 128], [72, KB2], [1, 4]]),
                    CLAMP_A, None, Alu.min)
                nc.scalar.activation(
                    vap(Ya, 68, [[KB2 * 72, 128], [72, KB2], [1, 4]]),
                    scl2[:], Act.Exp, bias=shiftB[:, 0:1])
                # Pre-expand the 4 per-head exp values to 8-wide on the
                # ScalarE so the payload multiply keeps stride-1 operands
                # (2x DVE mode); a [0,8]-broadcast operand would force 1x.
                nc.scalar.activation(
                    vap(ex82[:], 0,
                        [[KB2 * 64, 128], [64, KB2], [32, 2], [8, 4],
                         [1, 8]]),
                    vap(Ya, 32,
                        [[KB2 * 72, 128], [72, KB2], [36, 2], [1, 4],
                         [0, 8]]),
                    Act.Copy)
                nc.vector.tensor_tensor(
                    vap(Ya, 0,
                        [[KB2 * 72, 128], [72, KB2], [36, 2], [8, 4],
                         [1, 8]]),
                    vap(convV2[:], 0,
                        [[KB2 * 32, 128], [32, KB2], [0, 2], [8, 4],
                         [1, 8]]),
                    vap(ex82[:], 0,
                        [[KB2 * 64, 128], [64, KB2], [32, 2], [8, 4],
                         [1, 8]]),
                    Alu.mult)

                # ---- segment matmuls (one-hot from host), per bucket
                for h in range(2):
                    seg = spool.tile([128, 72], f32, tag="seg")
                    for i in range(KB):
                        nc.tensor.matmul(
                            out=seg[:],
                            lhsT=oh2[:, (h * KB + i) * 128:
                                     (h * KB + i + 1) * 128],
                            rhs=Y2[:, (h * KB + i) * 72:
                                   (h * KB + i + 1) * 72],
                            start=(i == 0), stop=(i == KB - 1))
                    nc.scalar.activation(
                        segS[:, (b0 + h) * 72:(b0 + h + 1) * 72], seg[:],
                        Act.Copy)

            # ---- software pipeline at PAIR granularity: the convs of the
            # previous pair are issued BEFORE this pair's MLP/tmp so the
            # bufs=2 pools rotate correctly (in-order engines make the
            # consumer reads precede the producer overwrites).
            prevp = None
            for k in range(NBk // 2):
                basis2, fu2, eft2, oh2 = load_pair(k)
                if prevp is not None:
                    pk, prw, ptmp2, poh2 = prevp
                    conv_bucket(2 * pk, prw[0], ptmp2)
                    conv_bucket(2 * pk + 1, prw[1], ptmp2)
                    scores_pair(2 * pk, poh2)
                rw_a = mlp_bucket(2 * k, eft2, 0)
                rw_c = mlp_bucket(2 * k + 1, eft2, KB * 128)
                tmp2 = tmp_pair(basis2, fu2)
                prevp = (k, (rw_a, rw_c), tmp2, oh2)
            pk, prw, ptmp2, poh2 = prevp
            conv_bucket(2 * pk, prw[0], ptmp2)
            conv_bucket(2 * pk + 1, prw[1], ptmp2)
            scores_pair(2 * pk, poh2)

            # ======== Phase 3: select pass, divide, project, store ========
            segA = segS[:]
            rdA = cpool.tile([128, NBk * 4], f32)
            nc.vector.tensor_scalar(
                rdA[:], vap(segA, 32, [[NBk * 72, 128], [72, NBk], [1, 4]]),
                1e-30, None, Alu.add)
            nc.vector.reciprocal(rdA[:], rdA[:])
            rdB = cpool.tile([128, NBk * 4], f32)
            nc.vector.tensor_scalar(
                rdB[:], vap(segA, 68, [[NBk * 72, 128], [72, NBk], [1, 4]]),
                1e-30, None, Alu.add)
            nc.vector.reciprocal(rdB[:], rdB[:])
            msk = cpool.tile([128, NBk * 4], f32)
            nc.vector.tensor_scalar(
                msk[:], vap(segA, 32, [[NBk * 72, 128], [72, NBk], [1, 4]]),
                SEL_TH, None, Alu.is_lt)
            oA = cpool.tile([128, NBk * 32], f32)
            nc.vector.tensor_tensor(
                vap(oA[:], 0, [[NBk * 32, 128], [32, NBk], [8, 4], [1, 8]]),
                vap(segA, 0, [[NBk * 72, 128], [72, NBk], [8, 4], [1, 8]]),
                vap(rdA[:], 0, [[NBk * 4, 128], [4, NBk], [1, 4], [0, 8]]),
                Alu.mult)
            oB = cpool.tile([128, NBk * 32], f32)
            nc.vector.tensor_tensor(
                vap(oB[:], 0, [[NBk * 32, 128], [32, NBk], [8, 4], [1, 8]]),
                vap(segA, 36, [[NBk * 72, 128], [72, NBk], [8, 4], [1, 8]]),
                vap(rdB[:], 0, [[NBk * 4, 128], [4, NBk], [1, 4], [0, 8]]),
                Alu.mult)
            osc = cpool.tile([128, NBk * 32], bf16)
            osca = osc[:]
            nc.vector.tensor_tensor(oA[:], oA[:], oB[:], Alu.subtract)
            nc.vector.tensor_tensor(
                vap(oA[:], 0, [[NBk * 32, 128], [32, NBk], [8, 4], [1, 8]]),
                vap(oA[:], 0, [[NBk * 32, 128], [32, NBk], [8, 4], [1, 8]]),
                vap(msk[:], 0, [[NBk * 4, 128], [4, NBk], [1, 4], [0, 8]]),
                Alu.mult)
            nc.vector.tensor_tensor(osc[:], oA[:], oB[:], Alu.add)
            res = cpool.tile([128, NBk * 32], bf16)
            resa = res[:]
            scr = cpool.tile([128, NBk * 32], bf16)
            scra = scr[:]
            for mp in range(8):
                tgt = resa if mp == 0 else scra
                nc.vector.tensor_tensor(
                    vap(tgt, 0, [[NBk * 32, 128], [32, NBk], [4, 8], [1, 4]]),
                    vap(osca, mp * 4,
                        [[NBk * 32, 128], [32, NBk], [0, 8], [1, 4]]),
                    vap(proj_t[:], mp * 32,
                        [[256, 128], [0, NBk], [4, 8], [1, 4]]),
                    Alu.mult)
                if mp > 0:
                    nc.vector.tensor_tensor(resa, resa, scra, Alu.add)
            nc.sync.dma_start(
                out=vap(out_d.ap(), 0, [[32, 128], [4096, NBk], [1, 32]]),
                in_=res[:])
    return nc


# ---------------------------------------------------------------------------
# Host-side prep
# ---------------------------------------------------------------------------
def _host_ln(features, ln_w, ln_b):
    f32 = np.float32
    feats = features.reshape(-1, M1, D1).astype(f32)
    onehot = np.eye(2, dtype=f32)[IX1]
    norms = np.sqrt((feats ** 2) @ onehot)
    x = norms.reshape(-1, 2, 8)
    mu = x.mean(-1, keepdims=True, dtype=f32).astype(f32)
    var = ((x - mu) ** 2).mean(-1, keepdims=True, dtype=f32).astype(f32)
    ln = (x - mu) / np.sqrt(var + LN_EPS) * ln_w + ln_b
    ln = np.maximum(ln, 0).astype(f32).reshape(-1, M1, 2)
    return (feats * (ln / (norms + EQ_EPS))[:, :, IX1]).astype(f32)


def _prep(inputs, cfg: Cfg = None):
    mdt = np.float32 if OPT["mlp_f32"] else np.float16
    tdt = np.float32 if OPT["tmp_f32"] else np.float16

    src = np.asarray(inputs["src"]).astype(np.int64)
    dst = np.asarray(inputs["dst"]).astype(np.int64)
    n_nodes = np.asarray(inputs["features"]).shape[0]
    # basis: (E, d1, pd) -> per-edge (pd, d1) so products are stride-1.
    # pd columns permuted from k=(p,d2) to pd_hat=(d2,p) so the device
    # tree's group order 8*pd_hat+m equals 32*d2 + (8p+m) = P0's layout.
    basis = np.asarray(inputs["basis"], np.float32).reshape(-1, 4, 16)
    pd_perm = np.array([p * 4 + d2 for d2 in range(4) for p in range(4)])
    basis = basis[:, :, pd_perm]
    basis = np.ascontiguousarray(basis.transpose(0, 2, 1)).reshape(-1, 64)
    ef = np.asarray(inputs["edge_feats"], np.float32)

    nb_l = cfg.nb if cfg is not None else NB
    nb_g = N_CORES * nb_l
    nodes_pad = nb_g * BUCKET_N

    deg = np.bincount(dst, minlength=nodes_pad)
    order = np.argsort(-deg, kind="stable")
    assign = np.empty(nodes_pad, dtype=np.int64)
    pos = np.empty(nodes_pad, dtype=np.int64)
    for r in range(BUCKET_N):
        sl = order[r * nb_g:(r + 1) * nb_g]
        buckets = np.arange(nb_g) if r % 2 == 0 else np.arange(nb_g)[::-1]
        assign[sl] = buckets
        pos[sl] = r
    loads = np.zeros(nb_g, dtype=np.int64)
    np.add.at(loads, assign[dst], 1)
    kb = int(math.ceil(loads.max() / 128.0))
    kb += kb & 1                       # half-bucket split needs even kb
    if cfg is None:
        cfg = Cfg(nb=nb_l, kb=kb)
    assert kb <= cfg.kb, f"kb={kb} exceeds cfg.kb={cfg.kb}"

    f = _host_ln(np.asarray(inputs["features"], np.float32),
                 np.asarray(inputs["ln_w"], np.float32),
                 np.asarray(inputs["ln_b"], np.float32))
    fU_all = f[src].reshape(-1, 32)

    eb = assign[dst]
    eorder = np.argsort(eb, kind="stable")
    bstart = np.searchsorted(eb[eorder], np.arange(nb_g + 1))

    E_PAD, CH, KB = cfg.e_pad, cfg.ch, cfg.kb
    arange128 = np.arange(128, dtype=np.int64)
    in_maps = []
    for core in range(N_CORES):
        basis_s = np.zeros((E_PAD, 64), np.float32)
        fu_s = np.zeros((E_PAD, 32), np.float32)
        eft_s = np.zeros((32, E_PAD), np.float32)
        dstrel_s = np.full((E_PAD,), -1, np.int64)
        for lb in range(cfg.nb):
            gb = core * cfg.nb + lb
            eidx = eorder[bstart[gb]:bstart[gb + 1]]
            n = len(eidx)
            assert n <= KB * 128
            o = lb * KB * 128
            basis_s[o:o + n] = basis[eidx]
            fu_s[o:o + n] = fU_all[eidx]
            eft_s[:, o:o + n] = ef[eidx].T
            dstrel_s[o:o + n] = pos[dst[eidx]]
        # bucket-block layouts: (NB, 128, KB*w); edge (chunk i, part p)
        basis_bb = (basis_s.reshape(cfg.nb, KB, 128, 64)
                    .transpose(0, 2, 1, 3).reshape(cfg.nb, 128, KB * 64))
        fu_bb = (fu_s.reshape(cfg.nb, KB, 128, 32)
                 .transpose(0, 2, 1, 3).reshape(cfg.nb, 128, KB * 32))
        # one-hot: (NB, KB, 128e) -> (NB, 128e, KB*128n)
        oh_np = (ml_dtypes.float8_e4m3 if OPT.get("oh_fp8") else BF16)
        oh = (dstrel_s.reshape(cfg.nb, KB, 128)[..., None] ==
              arange128).astype(oh_np)
        oh_bb = np.ascontiguousarray(
            oh.transpose(0, 2, 1, 3)).reshape(cfg.nb, 128, KB * 128)
        in_maps.append({
            "basis_s": np.ascontiguousarray(basis_bb).astype(tdt),
            "fu_s": np.ascontiguousarray(fu_bb).astype(tdt),
            "eft_s": eft_s.astype(mdt),
            "oh_s": oh_bb,
        })

    w1 = np.asarray(inputs["w1"], np.float32)
    b1 = np.asarray(inputs["b1"], np.float32).reshape(64, 1)
    w2 = np.asarray(inputs["w2"], np.float32)
    b2 = np.asarray(inputs["b2"], np.float32)
    w2s = w2.copy()
    b2s = b2.copy()
    w2s[0:256, :] *= SCALE          # k-rows: fold the attention scale in
    b2s[0:256] *= SCALE
    # rw column order (i, m, p) -> (i, p, m): matches tmp2's (p, m) inner
    # layout so P0's stride-1 inner dim pairs rw and tmp consistently.
    jp = np.array([p * 8 + m for m in range(8) for p in range(4)])
    rperm = (np.arange(24)[:, None] * 32 + jp[None, :]).ravel()
    inv = np.empty_like(rperm)
    inv[rperm] = np.arange(768)
    w2s = w2s[inv]
    b2s = b2s[inv]
    w2b = np.concatenate([w2s.T, b2s[None, :]], axis=0).astype(np.float32)
    projw = np.asarray(inputs["proj_w"], np.float32)
    ptbl_flat = np.zeros((256,), np.float32)
    for mpi in range(8):
        for m in range(8):
            for d in range(4):
                ptbl_flat[mpi * 32 + m * 4 + d] = projw[IX2[d] * 8 + m, mpi]
    ptbl = np.broadcast_to(ptbl_flat, (128, 256)).astype(BF16)
    for im in in_maps:
        im.update({
            "w1t_s": np.ascontiguousarray(w1.T).astype(mdt),
            "b1_s": b1,
            "w2b_s": w2b.astype(mdt),
            "proj_s": ptbl,
        })
    meta = {"assign": assign, "pos": pos, "n_nodes": n_nodes}
    return in_maps, meta, cfg


def _unshard(results, meta):
    out_cat = np.concatenate([r["out_s"] for r in results], axis=0)
    assign, pos, n = meta["assign"], meta["pos"], meta["n_nodes"]
    rows = assign[:n] * 128 + pos[:n]
    return out_cat[rows].reshape(n, M2, D2)


def _run(inputs, trace=False):
    _apply_patches()
    import concourse.bass as bass
    from concourse.bass_utils import run_bass_kernel_spmd

    in_maps, meta, cfg = _prep(inputs)
    nc = bass.Bass("TRN2", target_bir_lowering=False)
    build_kernel(nc, cfg)
    r = run_bass_kernel_spmd(nc, in_maps, core_ids=list(range(N_CORES)),
                             trace=trace)
    out = _unshard(r.results, meta)
    return out, r


def kernel(**inputs) -> np.ndarray:
    out, _ = _run(inputs, trace=False)
    return out.astype(np.float32)



# revision 22
# speedup vs baseline: 1.0030x; 1.0030x over previous
"""Trainium2 Bass kernel for nn_EquivariantTransformerBlock.

Strategy (8 NeuronCores, no collectives, no indirect DMA):
  - Host assigns each node to one of 320 buckets of 128 nodes (degree-
    balanced snake packing); every edge goes to the core owning its dst
    bucket, so all segment sums are core-local.
  - Host computes the (tiny) equivariant LayerNorm, gathers f[src], and
    precomputes per-chunk one-hot matrices (fp8) for the segment sums.
  - Device pipeline, bucket (KB x 128 edges) at a time, software-
    pipelined one bucket ahead (PE MLP + ScalarE casts + DMA run under
    the DVE cascade of the previous bucket):
      * edge MLP on the TensorE in fp16; PSUM->SBUF casts on ScalarE,
      * fU x basis and rw x tmp contractions as fp16 broadcast-product
        + halving-tree ops on the VectorE (fp16 keeps the DVE 2x mode
        with 4x finer rounding than bf16; scores need that precision),
      * attention scores with the 1/sqrt(h) scale folded into the
        k-rows of w2 on the host,
      * dual-exp softmax (shift-free: clamped exp(s) / exp(s-140) with
        per-node select in the epilogue) with bf16 payload range,
      * segment sums as one-hot matmuls accumulated in PSUM per bucket.

DVE scheduling rules this kernel is tuned around (hardware-measured):
  - tensor_tensor runs 2x (2 elem/cyc/lane) only with 16-bit dtypes and
    unit-stride, 4B-aligned innermost dims on ALL operands; ~151 cycles
    fixed cost per instruction.
  - tensor_reduce has only a 1x uop -> the d1-sum over the fU x basis
    products is a 2-level halving tree instead (L1 at 2x); the host
    permutes basis cols to pd=(d2,p) and w2 rows to j=(p,m) so every
    tree level writes unit-stride (strided writes cost ~4.3 cyc/elem).
  - the per-head exp values are pre-expanded 8-wide on the ScalarE so
    the payload multiply keeps 2x (a [0,8]-broadcast operand forces 1x).
  - score/softmax/payload ops run once per bucket PAIR (doubled FD,
    halved fixed cost); ScalarE Lrelu is avoided (ACT_TABLE_LOAD thrash
    + wrong alpha on HW).
"""

import math
from contextlib import ExitStack
from dataclasses import dataclass

import numpy as np
import ml_dtypes

BF16 = ml_dtypes.bfloat16

N_NODES = 40000
N_EDGES = 320000
M1, D1 = 8, 4
M2, D2 = 8, 4
LN_EPS = 1e-5
EQ_EPS = 1e-8
IX1 = np.array([0, 1, 1, 1])
IX2 = np.array([0, 1, 1, 1])

N_CORES = 8
BUCKET_N = 128
NB = 40
SCALE = 32.0 ** -0.5
SHIFT_B = 140.0
CLAMP_A = 1e34
SEL_TH = 1e33

# Precision knobs (validated against numpy sim of the same pipeline):
#   mlp_f32: run the edge MLP matmuls in fp32
#   tmp_f32: run the fU x basis cascade in fp32 (quantize tmp2 to bf16)
OPT = {"mlp_f32": False, "tmp_f32": False, "tmp_gpsimd": False, "tmp_5d": False, "oh_fp8": True}


@dataclass
class Cfg:
    nb: int
    kb: int

    @property
    def ch(self):
        return self.nb * self.kb

    @property
    def e_pad(self):
        return self.ch * 128


# ---------------------------------------------------------------------------
# Patches: this walrus build allows at most ONE sync wait per instruction.
# ---------------------------------------------------------------------------
_PATCHED = False


def _apply_patches():
    global _PATCHED
    if _PATCHED:
        return
    _PATCHED = True
    import re as _re

    import orjson as _orjson

    import concourse.bass as _bass
    from concourse.tile import TileContext as _TC
    from concourse.vector_clock import ScopedClock as _SC, VectorClock as _VC

    def _drain_and_barrier(self, tick_clock, wait_clock):
        nc = self.nc
        gvals = [int(x) for x in _re.findall(r"\d+", repr(tick_clock.global_clock))]
        nz = [(p, v) for p, v in enumerate(gvals) if v > 0]
        if not nz:
            nc.sync.drain()
        for p, v in nz:
            pvc = _VC()
            pvc.require_at_least(p, v)
            d = nc.sync.drain()
            wait_clock.add_sem_waits(d.ins, _SC({None: pvc}))
        nc.all_engine_barrier()
        assert self.sems is not None
        popped = nc._tile_sem_poison_stack.pop()
        assert popped is self._sem_poison
        nc.clear_and_free_semaphores(list(self.sems.allocated().values()))
        nc.all_engine_barrier()

    def _split_multi_waits(data: bytes) -> bytes:
        j = _orjson.loads(data)
        for fn in j.get("functions", []):
            for bb in fn.get("blocks", []):
                out = []
                for ins in bb.get("instructions", []):
                    si = ins.get("sync_info")
                    ow = (si or {}).get("on_wait") or []
                    if len(ow) > 1:
                        for k, w in enumerate(ow[:-1]):
                            out.append({
                                "debug": ins.get("debug", 0),
                                "engine": ins["engine"],
                                "ins": [],
                                "name": f"{ins['name']}-spw{k}",
                                "opcode": "EventSemaphore",
                                "outs": [],
                                "sync_info": {"on_update": [], "on_wait": [w]},
                            })
                        si["on_wait"] = [ow[-1]]
                    out.append(ins)
                bb["instructions"] = out
        return _orjson.dumps(j)

    _orig_to_json_bytes = _bass.Bass.to_json_bytes

    def _to_json_bytes(self):
        return _split_multi_waits(_orig_to_json_bytes(self))

    _TC._drain_and_barrier = _drain_and_barrier
    _bass.Bass.to_json_bytes = _to_json_bytes


# ---------------------------------------------------------------------------
# Device kernel
# ---------------------------------------------------------------------------
def build_kernel(nc, cfg: Cfg):
    import concourse.bass as bass
    import concourse.mybir as mybir
    from concourse.tile import TileContext

    f32 = mybir.dt.float32
    bf16 = mybir.dt.bfloat16
    fp16 = mybir.dt.float16
    Alu = mybir.AluOpType
    Act = mybir.ActivationFunctionType

    NBk, KB, CH, E_PAD = cfg.nb, cfg.kb, cfg.ch, cfg.e_pad
    HB = KB // 2            # half-bucket chunk count for the big cascade
    mdt = f32 if OPT["mlp_f32"] else fp16
    tdt = f32 if OPT["tmp_f32"] else fp16
    cdt = fp16

    basis_d = nc.dram_tensor("basis_s", (NBk, 128, KB * 64), tdt,
                             kind="ExternalInput")
    fu_d = nc.dram_tensor("fu_s", (NBk, 128, KB * 32), tdt,
                          kind="ExternalInput")
    eft_d = nc.dram_tensor("eft_s", (32, E_PAD), mdt, kind="ExternalInput")
    fp8 = mybir.dt.float8e4
    ohdt = fp8 if OPT.get("oh_fp8") else bf16
    oh_d = nc.dram_tensor("oh_s", (NBk, 128, KB * 128), ohdt,
                          kind="ExternalInput")
    w1t_d = nc.dram_tensor("w1t_s", (32, 64), mdt, kind="ExternalInput")
    b1_d = nc.dram_tensor("b1_s", (64, 1), f32, kind="ExternalInput")
    w2b_d = nc.dram_tensor("w2b_s", (65, 768), mdt, kind="ExternalInput")
    proj_d = nc.dram_tensor("proj_s", (128, 256), bf16, kind="ExternalInput")
    out_d = nc.dram_tensor("out_s", (NBk * 128, 32), bf16,
                           kind="ExternalOutput")

    def vap(base, offset, dims):
        return bass.AP(base.tensor, base.offset + offset, dims)

    with TileContext(nc) as tc:
        with ExitStack() as ctx:
            cpool = ctx.enter_context(tc.tile_pool(name="consts", bufs=1))
            w1t_t = cpool.tile([32, 64], mdt)
            nc.sync.dma_start(out=w1t_t[:], in_=w1t_d.ap())
            b1_t = cpool.tile([64, 1], f32)
            nc.sync.dma_start(out=b1_t[:], in_=b1_d.ap())
            w2b_t = cpool.tile([65, 768], mdt)
            nc.sync.dma_start(out=w2b_t[:], in_=w2b_d.ap())
            proj_t = cpool.tile([128, 256], bf16)
            nc.sync.dma_start(out=proj_t[:], in_=proj_d.ap())
            segS = cpool.tile([128, NBk * 72], f32)
            shiftB = cpool.tile([128, 1], f32)
            nc.vector.memset(shiftB[:], -SHIFT_B)

            bpool = ctx.enter_context(tc.tile_pool(name="edges", bufs=3))
            rpool = ctx.enter_context(tc.tile_pool(name="rw", bufs=2))
            gpool = ctx.enter_context(tc.tile_pool(name="tmp", bufs=2))
            tpool = ctx.enter_context(tc.tile_pool(name="work", bufs=1))
            hpool = ctx.enter_context(
                tc.tile_pool(name="psH", bufs=2, space="PSUM"))
            ppool = ctx.enter_context(
                tc.tile_pool(name="psR", bufs=2, space="PSUM"))
            spool = ctx.enter_context(
                tc.tile_pool(name="psS", bufs=2, space="PSUM"))

            # persistent h65 pair with the ones-row set once
            h65s = [cpool.tile([65, 128], mdt, name=f"h65p{k}")
                    for k in range(2)]
            for t in h65s:
                nc.gpsimd.memset(t[64:65, :], 1.0)

            def load_bucket(b):
                basis_b = bpool.tile([128, KB * 64], tdt, tag="basisb")
                nc.sync.dma_start(
                    out=basis_b[:],
                    in_=vap(basis_d.ap(), b * 128 * KB * 64,
                            [[KB * 64, 128], [1, KB * 64]]))
                fu_b = bpool.tile([128, KB * 32], tdt, tag="fub")
                nc.sync.dma_start(
                    out=fu_b[:],
                    in_=vap(fu_d.ap(), b * 128 * KB * 32,
                            [[KB * 32, 128], [1, KB * 32]]))
                eft_b = bpool.tile([32, KB * 128], mdt, tag="eftb")
                nc.sync.dma_start(
                    out=eft_b[:],
                    in_=vap(eft_d.ap(), b * KB * 128,
                            [[E_PAD, 32], [1, KB * 128]]))
                oh_b = bpool.tile([128, KB * 128], ohdt, tag="ohb")
                nc.sync.dma_start(
                    out=oh_b[:],
                    in_=vap(oh_d.ap(), b * 128 * KB * 128,
                            [[KB * 128, 128], [1, KB * 128]]))
                return basis_b, fu_b, eft_b, oh_b

            def mlp_bucket(b, eft_b):
                rwb = rpool.tile([128, KB * 768], cdt, tag="rwb")
                for i in range(KB):
                    hps = hpool.tile([64, 128], f32, tag="hps")
                    nc.tensor.matmul(out=hps[:], lhsT=w1t_t[:],
                                     rhs=eft_b[:, i * 128:(i + 1) * 128],
                                     start=True, stop=True)
                    h65 = h65s[i % 2]
                    nc.scalar.activation(h65[0:64, :], hps[:], Act.Relu,
                                         bias=b1_t[:, 0:1])
                    rwp = ppool.tile([128, 768], f32, tag="rwp")
                    nc.tensor.matmul(out=rwp[:, 0:512], lhsT=h65[:],
                                     rhs=w2b_t[:, 0:512], start=True,
                                     stop=True)
                    nc.tensor.matmul(out=rwp[:, 512:768], lhsT=h65[:],
                                     rhs=w2b_t[:, 512:768], start=True,
                                     stop=True)
                    nc.scalar.activation(rwb[:, i * 768:(i + 1) * 768],
                                         rwp[:], Act.Copy)
                return rwb

            def tmp_bucket(b, basis_b, fu_b):
                """fU x basis products + d1-sum -> tmp2 in (d2, j) layout.

                The d1-sum is a 2-level halving tree instead of a
                tensor_reduce: reduce only has a 1x uop (FD cycles), while
                the first tree level runs at 2x (pairs (d1,d1+2) keep both
                operands stride-1 / 4B-aligned). L2 pairs are stride-2 ->
                1x, but its FD is only a quarter of the reduce's.
                """
                tmp2 = gpool.tile([128, KB * 128], cdt, tag="tmp2")
                tmp2a = tmp2[:]
                ptb = gpool.tile([128, KB * 512], tdt, tag="ptb")
                ptba = ptb[:]
                # ptb layout per edge: [pd(16):32][m(8):4][d1(4):1]; the host
                # permutes basis cols to pd=(d2,p) and w2 rows to j=(p,m), so
                # the tree's natural group order 8*pd+m lands tmp2 directly
                # in P0's [d2:32][(p,m):1] layout -- every tree op then has a
                # unit-stride output (strided DVE writes cost ~4.3 cyc/elem).
                nc.vector.tensor_tensor(
                    vap(ptba, 0,
                        [[KB * 512, 128], [512, KB], [32, 16], [4, 8],
                         [1, 4]]),
                    vap(fu_b[:], 0,
                        [[KB * 32, 128], [32, KB], [0, 16], [4, 8],
                         [1, 4]]),
                    vap(basis_b[:], 0,
                        [[KB * 64, 128], [64, KB], [4, 16], [0, 8],
                         [1, 4]]),
                    Alu.mult)
                h1 = tpool.tile([128, KB * 256], cdt, tag="h1")
                h1a = h1[:]
                with nc.allow_low_precision(reason="fp16 tmp"):
                    nc.vector.tensor_tensor(
                        vap(h1a, 0,
                            [[KB * 256, 128], [256, KB], [2, 128], [1, 2]]),
                        vap(ptba, 0,
                            [[KB * 512, 128], [512, KB], [4, 128], [1, 2]]),
                        vap(ptba, 2,
                            [[KB * 512, 128], [512, KB], [4, 128], [1, 2]]),
                        Alu.add)
                    nc.vector.tensor_tensor(
                        vap(tmp2a, 0,
                            [[KB * 128, 128], [128, KB], [1, 128]]),
                        vap(h1a, 0,
                            [[KB * 256, 128], [256, KB], [2, 128]]),
                        vap(h1a, 1,
                            [[KB * 256, 128], [256, KB], [2, 128]]),
                        Alu.add)
                return tmp2

            # Pair-level tiles: the cascade fills half-slots per bucket and
            # the score/payload ops run once per 2 buckets with doubled FD,
            # halving their fixed per-instruction cost (~151 DVE cycles).
            KB2 = 2 * KB
            conv2 = tpool.tile([128, KB2 * 64], cdt, name="conv2")
            convV2 = tpool.tile([128, KB2 * 32], bf16, name="convV2")
            Y2 = tpool.tile([128, KB2 * 72], bf16, name="Y2")
            ex82 = tpool.tile([128, KB2 * 64], bf16, name="ex82")
            ps2 = tpool.tile([128, KB2 * 32], cdt, name="ps2")
            sc42 = tpool.tile([128, KB2 * 4], f32, name="sc42")
            scl2 = tpool.tile([128, KB2 * 4], f32, name="scl2")

            def conv_bucket(b, rwb, tmp2):
                rwba = rwb[:]
                tmp2a = tmp2[:]
                half = (b & 1) * KB
                P0 = tpool.tile([128, KB * 3072], cdt, tag="P0")
                T1 = tpool.tile([128, KB * 1536], cdt, tag="T1")
                T2 = tpool.tile([128, KB * 768], cdt, tag="T2")
                T3 = tpool.tile([128, KB * 384], cdt, tag="T3")
                T4 = tpool.tile([128, KB * 192], cdt, tag="T4")
                nc.vector.tensor_tensor(
                    vap(P0[:], 0,
                        [[KB * 3072, 128], [3072, KB], [128, 24],
                         [32, 4], [1, 32]]),
                    vap(rwba, 0,
                        [[KB * 768, 128], [768, KB], [32, 24], [0, 4],
                         [1, 32]]),
                    vap(tmp2a, 0,
                        [[KB * 128, 128], [128, KB], [0, 24], [32, 4],
                         [1, 32]]),
                    Alu.mult)
                cur, w = P0[:], 32
                nxts = {16: T1, 8: T2, 4: T3, 2: T4}
                while w > 2:
                    w2 = w // 2
                    nxt = nxts[w2]
                    nc.vector.tensor_tensor(
                        vap(nxt[:], 0,
                            [[KB * 96 * w2, 128], [96 * w2, KB],
                             [4 * w2, 24], [w2, 4], [1, w2]]),
                        vap(cur, 0,
                            [[KB * 96 * w, 128], [96 * w, KB],
                             [4 * w, 24], [w, 4], [1, w2]]),
                        vap(cur, w2,
                            [[KB * 96 * w, 128], [96 * w, KB],
                             [4 * w, 24], [w, 4], [1, w2]]),
                        Alu.add)
                    cur, w = nxt[:], w2
                nc.vector.tensor_tensor(
                    vap(conv2[:], half * 64,
                        [[KB2 * 64, 128], [64, KB], [4, 16], [1, 4]]),
                    vap(cur, 0,
                        [[KB * 192, 128], [192, KB], [8, 16], [2, 4]]),
                    vap(cur, 1,
                        [[KB * 192, 128], [192, KB], [8, 16], [2, 4]]),
                    Alu.add)
                nc.vector.tensor_tensor(
                    vap(convV2[:], half * 32,
                        [[KB2 * 32, 128], [32, KB], [4, 8], [1, 4]]),
                    vap(cur, 128,
                        [[KB * 192, 128], [192, KB], [8, 8], [2, 4]]),
                    vap(cur, 129,
                        [[KB * 192, 128], [192, KB], [8, 8], [2, 4]]),
                    Alu.add)

            def scores_pair(b0, oh_pair):
                """Scores + dual-exp softmax + payload for buckets b0,b0+1."""
                Ya = Y2[:]
                nc.vector.tensor_tensor(
                    ps2[:],
                    vap(conv2[:], 0,
                        [[KB2 * 64, 128], [64, KB2], [1, 32]]),
                    vap(conv2[:], 32,
                        [[KB2 * 64, 128], [64, KB2], [1, 32]]),
                    Alu.mult)
                nc.vector.tensor_reduce(
                    sc42[:],
                    vap(ps2[:], 0,
                        [[KB2 * 32, 128], [8, KB2 * 4], [1, 8]]),
                    mybir.AxisListType.X, Alu.add)
                # SCALE is folded into the k-rows of w2 on the host.
                # (ScalarE Lrelu was tried here: it thrashes the activation
                # table -- 1.3us ACT_TABLE_LOAD per swap vs Exp -- and its
                # alpha lowering looked wrong on HW. Keep leaky on DVE.)
                nc.vector.scalar_tensor_tensor(
                    scl2[:], sc42[:], 0.2, sc42[:], Alu.mult, Alu.max)
                nc.scalar.activation(
                    vap(Ya, 32, [[KB2 * 72, 128], [72, KB2], [1, 4]]),
                    scl2[:], Act.Exp)
                nc.vector.tensor_scalar(
                    vap(Ya, 32, [[KB2 * 72, 128], [72, KB2], [1, 4]]),
                    vap(Ya, 32, [[KB2 * 72, 128], [72, KB2], [1, 4]]),
                    CLAMP_A, None, Alu.min)
                nc.scalar.activation(
                    vap(Ya, 68, [[KB2 * 72, 128], [72, KB2], [1, 4]]),
                    scl2[:], Act.Exp, bias=shiftB[:, 0:1])
                # Pre-expand the 4 per-head exp values to 8-wide on the
                # ScalarE so the payload multiply keeps stride-1 operands
                # (2x DVE mode); a [0,8]-broadcast operand would force 1x.
                nc.scalar.activation(
                    vap(ex82[:], 0,
                        [[KB2 * 64, 128], [64, KB2], [32, 2], [8, 4],
                         [1, 8]]),
                    vap(Ya, 32,
                        [[KB2 * 72, 128], [72, KB2], [36, 2], [1, 4],
                         [0, 8]]),
                    Act.Copy)
                nc.vector.tensor_tensor(
                    vap(Ya, 0,
                        [[KB2 * 72, 128], [72, KB2], [36, 2], [8, 4],
                         [1, 8]]),
                    vap(convV2[:], 0,
                        [[KB2 * 32, 128], [32, KB2], [0, 2], [8, 4],
                         [1, 8]]),
                    vap(ex82[:], 0,
                        [[KB2 * 64, 128], [64, KB2], [32, 2], [8, 4],
                         [1, 8]]),
                    Alu.mult)

                # ---- segment matmuls (one-hot from host), per bucket
                for h in range(2):
                    seg = spool.tile([128, 72], f32, tag="seg")
                    for i in range(KB):
                        nc.tensor.matmul(
                            out=seg[:],
                            lhsT=oh_pair[h][:, i * 128:(i + 1) * 128],
                            rhs=Y2[:, (h * KB + i) * 72:
                                   (h * KB + i + 1) * 72],
                            start=(i == 0), stop=(i == KB - 1))
                    nc.scalar.activation(
                        segS[:, (b0 + h) * 72:(b0 + h + 1) * 72], seg[:],
                        Act.Copy)

            # ---- software pipeline: MLP + tmp run one bucket ahead
            prev = None
            oh_even = None
            for b in range(NBk):
                basis_b, fu_b, eft_b, oh_b = load_bucket(b)
                rwb = mlp_bucket(b, eft_b)
                tmp2 = tmp_bucket(b, basis_b, fu_b)
                if prev is not None:
                    pb, prwb, ptmp2, poh = prev
                    conv_bucket(pb, prwb, ptmp2)
                    if pb & 1:
                        scores_pair(pb - 1, (oh_even, poh))
                    else:
                        oh_even = poh
                prev = (b, rwb, tmp2, oh_b)
            pb, prwb, ptmp2, poh = prev
            conv_bucket(pb, prwb, ptmp2)
            scores_pair(pb - 1, (oh_even, poh))

            # ======== Phase 3: select pass, divide, project, store ========
            segA = segS[:]
            rdA = cpool.tile([128, NBk * 4], f32)
            nc.vector.tensor_scalar(
                rdA[:], vap(segA, 32, [[NBk * 72, 128], [72, NBk], [1, 4]]),
                1e-30, None, Alu.add)
            nc.vector.reciprocal(rdA[:], rdA[:])
            rdB = cpool.tile([128, NBk * 4], f32)
            nc.vector.tensor_scalar(
                rdB[:], vap(segA, 68, [[NBk * 72, 128], [72, NBk], [1, 4]]),
                1e-30, None, Alu.add)
            nc.vector.reciprocal(rdB[:], rdB[:])
            msk = cpool.tile([128, NBk * 4], f32)
            nc.vector.tensor_scalar(
                msk[:], vap(segA, 32, [[NBk * 72, 128], [72, NBk], [1, 4]]),
                SEL_TH, None, Alu.is_lt)
            oA = cpool.tile([128, NBk * 32], f32)
            nc.vector.tensor_tensor(
                vap(oA[:], 0, [[NBk * 32, 128], [32, NBk], [8, 4], [1, 8]]),
                vap(segA, 0, [[NBk * 72, 128], [72, NBk], [8, 4], [1, 8]]),
                vap(rdA[:], 0, [[NBk * 4, 128], [4, NBk], [1, 4], [0, 8]]),
                Alu.mult)
            oB = cpool.tile([128, NBk * 32], f32)
            nc.vector.tensor_tensor(
                vap(oB[:], 0, [[NBk * 32, 128], [32, NBk], [8, 4], [1, 8]]),
                vap(segA, 36, [[NBk * 72, 128], [72, NBk], [8, 4], [1, 8]]),
                vap(rdB[:], 0, [[NBk * 4, 128], [4, NBk], [1, 4], [0, 8]]),
                Alu.mult)
            osc = cpool.tile([128, NBk * 32], bf16)
            osca = osc[:]
            nc.vector.tensor_tensor(oA[:], oA[:], oB[:], Alu.subtract)
            nc.vector.tensor_tensor(
                vap(oA[:], 0, [[NBk * 32, 128], [32, NBk], [8, 4], [1, 8]]),
                vap(oA[:], 0, [[NBk * 32, 128], [32, NBk], [8, 4], [1, 8]]),
                vap(msk[:], 0, [[NBk * 4, 128], [4, NBk], [1, 4], [0, 8]]),
                Alu.mult)
            nc.vector.tensor_tensor(osc[:], oA[:], oB[:], Alu.add)
            res = cpool.tile([128, NBk * 32], bf16)
            resa = res[:]
            scr = cpool.tile([128, NBk * 32], bf16)
            scra = scr[:]
            for mp in range(8):
                tgt = resa if mp == 0 else scra
                nc.vector.tensor_tensor(
                    vap(tgt, 0, [[NBk * 32, 128], [32, NBk], [4, 8], [1, 4]]),
                    vap(osca, mp * 4,
                        [[NBk * 32, 128], [32, NBk], [0, 8], [1, 4]]),
                    vap(proj_t[:], mp * 32,
                        [[256, 128], [0, NBk], [4, 8], [1, 4]]),
                    Alu.mult)
                if mp > 0:
                    nc.vector.tensor_tensor(resa, resa, scra, Alu.add)
            nc.sync.dma_start(
                out=vap(out_d.ap(), 0, [[32, 128], [4096, NBk], [1, 32]]),
                in_=res[:])
    return nc


# ---------------------------------------------------------------------------
# Host-side prep
# ---------------------------------------------------------------------------
def _host_ln(features, ln_w, ln_b):
    f32 = np.float32
    feats = features.reshape(-1, M1, D1).astype(f32)
    onehot = np.eye(2, dtype=f32)[IX1]
    norms = np.sqrt((feats ** 2) @ onehot)
    x = norms.reshape(-1, 2, 8)
    mu = x.mean(-1, keepdims=True, dtype=f32).astype(f32)
    var = ((x - mu) ** 2).mean(-1, keepdims=True, dtype=f32).astype(f32)
    ln = (x - mu) / np.sqrt(var + LN_EPS) * ln_w + ln_b
    ln = np.maximum(ln, 0).astype(f32).reshape(-1, M1, 2)
    return (feats * (ln / (norms + EQ_EPS))[:, :, IX1]).astype(f32)


def _prep(inputs, cfg: Cfg = None):
    mdt = np.float32 if OPT["mlp_f32"] else np.float16
    tdt = np.float32 if OPT["tmp_f32"] else np.float16

    src = np.asarray(inputs["src"]).astype(np.int64)
    dst = np.asarray(inputs["dst"]).astype(np.int64)
    n_nodes = np.asarray(inputs["features"]).shape[0]
    # basis: (E, d1, pd) -> per-edge (pd, d1) so products are stride-1.
    # pd columns permuted from k=(p,d2) to pd_hat=(d2,p) so the device
    # tree's group order 8*pd_hat+m equals 32*d2 + (8p+m) = P0's layout.
    basis = np.asarray(inputs["basis"], np.float32).reshape(-1, 4, 16)
    pd_perm = np.array([p * 4 + d2 for d2 in range(4) for p in range(4)])
    basis = basis[:, :, pd_perm]
    basis = np.ascontiguousarray(basis.transpose(0, 2, 1)).reshape(-1, 64)
    ef = np.asarray(inputs["edge_feats"], np.float32)

    nb_l = cfg.nb if cfg is not None else NB
    nb_g = N_CORES * nb_l
    nodes_pad = nb_g * BUCKET_N

    deg = np.bincount(dst, minlength=nodes_pad)
    order = np.argsort(-deg, kind="stable")
    assign = np.empty(nodes_pad, dtype=np.int64)
    pos = np.empty(nodes_pad, dtype=np.int64)
    for r in range(BUCKET_N):
        sl = order[r * nb_g:(r + 1) * nb_g]
        buckets = np.arange(nb_g) if r % 2 == 0 else np.arange(nb_g)[::-1]
        assign[sl] = buckets
        pos[sl] = r
    loads = np.zeros(nb_g, dtype=np.int64)
    np.add.at(loads, assign[dst], 1)
    kb = int(math.ceil(loads.max() / 128.0))
    kb += kb & 1                       # half-bucket split needs even kb
    if cfg is None:
        cfg = Cfg(nb=nb_l, kb=kb)
    assert kb <= cfg.kb, f"kb={kb} exceeds cfg.kb={cfg.kb}"

    f = _host_ln(np.asarray(inputs["features"], np.float32),
                 np.asarray(inputs["ln_w"], np.float32),
                 np.asarray(inputs["ln_b"], np.float32))
    fU_all = f[src].reshape(-1, 32)

    eb = assign[dst]
    eorder = np.argsort(eb, kind="stable")
    bstart = np.searchsorted(eb[eorder], np.arange(nb_g + 1))

    E_PAD, CH, KB = cfg.e_pad, cfg.ch, cfg.kb
    arange128 = np.arange(128, dtype=np.int64)
    in_maps = []
    for core in range(N_CORES):
        basis_s = np.zeros((E_PAD, 64), np.float32)
        fu_s = np.zeros((E_PAD, 32), np.float32)
        eft_s = np.zeros((32, E_PAD), np.float32)
        dstrel_s = np.full((E_PAD,), -1, np.int64)
        for lb in range(cfg.nb):
            gb = core * cfg.nb + lb
            eidx = eorder[bstart[gb]:bstart[gb + 1]]
            n = len(eidx)
            assert n <= KB * 128
            o = lb * KB * 128
            basis_s[o:o + n] = basis[eidx]
            fu_s[o:o + n] = fU_all[eidx]
            eft_s[:, o:o + n] = ef[eidx].T
            dstrel_s[o:o + n] = pos[dst[eidx]]
        # bucket-block layouts: (NB, 128, KB*w); edge (chunk i, part p)
        basis_bb = (basis_s.reshape(cfg.nb, KB, 128, 64)
                    .transpose(0, 2, 1, 3).reshape(cfg.nb, 128, KB * 64))
        fu_bb = (fu_s.reshape(cfg.nb, KB, 128, 32)
                 .transpose(0, 2, 1, 3).reshape(cfg.nb, 128, KB * 32))
        # one-hot: (NB, KB, 128e) -> (NB, 128e, KB*128n)
        oh_np = (ml_dtypes.float8_e4m3 if OPT.get("oh_fp8") else BF16)
        oh = (dstrel_s.reshape(cfg.nb, KB, 128)[..., None] ==
              arange128).astype(oh_np)
        oh_bb = np.ascontiguousarray(
            oh.transpose(0, 2, 1, 3)).reshape(cfg.nb, 128, KB * 128)
        in_maps.append({
            "basis_s": np.ascontiguousarray(basis_bb).astype(tdt),
            "fu_s": np.ascontiguousarray(fu_bb).astype(tdt),
            "eft_s": eft_s.astype(mdt),
            "oh_s": oh_bb,
        })

    w1 = np.asarray(inputs["w1"], np.float32)
    b1 = np.asarray(inputs["b1"], np.float32).reshape(64, 1)
    w2 = np.asarray(inputs["w2"], np.float32)
    b2 = np.asarray(inputs["b2"], np.float32)
    w2s = w2.copy()
    b2s = b2.copy()
    w2s[0:256, :] *= SCALE          # k-rows: fold the attention scale in
    b2s[0:256] *= SCALE
    # rw column order (i, m, p) -> (i, p, m): matches tmp2's (p, m) inner
    # layout so P0's stride-1 inner dim pairs rw and tmp consistently.
    jp = np.array([p * 8 + m for m in range(8) for p in range(4)])
    rperm = (np.arange(24)[:, None] * 32 + jp[None, :]).ravel()
    inv = np.empty_like(rperm)
    inv[rperm] = np.arange(768)
    w2s = w2s[inv]
    b2s = b2s[inv]
    w2b = np.concatenate([w2s.T, b2s[None, :]], axis=0).astype(np.float32)
    projw = np.asarray(inputs["proj_w"], np.float32)
    ptbl_flat = np.zeros((256,), np.float32)
    for mpi in range(8):
        for m in range(8):
            for d in range(4):
                ptbl_flat[mpi * 32 + m * 4 + d] = projw[IX2[d] * 8 + m, mpi]
    ptbl = np.broadcast_to(ptbl_flat, (128, 256)).astype(BF16)
    for im in in_maps:
        im.update({
            "w1t_s": np.ascontiguousarray(w1.T).astype(mdt),
            "b1_s": b1,
            "w2b_s": w2b.astype(mdt),
            "proj_s": ptbl,
        })
    meta = {"assign": assign, "pos": pos, "n_nodes": n_nodes}
    return in_maps, meta, cfg


def _unshard(results, meta):
    out_cat = np.concatenate([r["out_s"] for r in results], axis=0)
    assign, pos, n = meta["assign"], meta["pos"], meta["n_nodes"]
    rows = assign[:n] * 128 + pos[:n]
    return out_cat[rows].reshape(n, M2, D2)


def _run(inputs, trace=False):
    _apply_patches()
    import concourse.bass as bass
    from concourse.bass_utils import run_bass_kernel_spmd

    in_maps, meta, cfg = _prep(inputs)
    nc = bass.Bass("TRN2", target_bir_lowering=False)
    build_kernel(nc, cfg)
    r = run_bass_kernel_spmd(nc, in_maps, core_ids=list(range(N_CORES)),
                             trace=trace)
    out = _unshard(r.results, meta)
    return out, r


def kernel(**inputs) -> np.ndarray:
    out, _ = _run(inputs, trace=False)
    return out.astype(np.float32)



# revision 28
# speedup vs baseline: 1.0144x; 1.0113x over previous
"""Trainium2 Bass kernel for nn_EquivariantTransformerBlock.

Strategy (8 NeuronCores, no collectives, no indirect DMA):
  - Host assigns each node to one of 320 buckets of 128 nodes (degree-
    balanced snake packing); every edge goes to the core owning its dst
    bucket, so all segment sums are core-local.
  - Host computes the (tiny) equivariant LayerNorm, gathers f[src], and
    precomputes per-chunk one-hot matrices (fp8) for the segment sums.
  - Device pipeline, bucket (KB x 128 edges) at a time, software-
    pipelined one bucket ahead (PE MLP + ScalarE casts + DMA run under
    the DVE cascade of the previous bucket):
      * edge MLP on the TensorE in fp16; PSUM->SBUF casts on ScalarE,
      * fU x basis and rw x tmp contractions as fp16 broadcast-product
        + halving-tree ops on the VectorE (fp16 keeps the DVE 2x mode
        with 4x finer rounding than bf16; scores need that precision),
      * attention scores with the 1/sqrt(h) scale folded into the
        k-rows of w2 on the host,
      * dual-exp softmax (shift-free: clamped exp(s) / exp(s-140) with
        per-node select in the epilogue) with bf16 payload range,
      * segment sums as one-hot matmuls accumulated in PSUM per bucket.

DVE scheduling rules this kernel is tuned around (hardware-measured):
  - tensor_tensor runs 2x (2 elem/cyc/lane) only with 16-bit dtypes and
    unit-stride, 4B-aligned innermost dims on ALL operands; ~151 cycles
    fixed cost per instruction.
  - tensor_reduce has only a 1x uop -> the d1-sum over the fU x basis
    products is a 2-level halving tree instead (L1 at 2x); the host
    permutes basis cols to pd=(d2,p) and w2 rows to j=(p,m) so every
    tree level writes unit-stride (strided writes cost ~4.3 cyc/elem).
  - the per-head exp values are pre-expanded 8-wide on the ScalarE so
    the payload multiply keeps 2x (a [0,8]-broadcast operand forces 1x).
  - score/softmax/payload ops run once per bucket PAIR (doubled FD,
    halved fixed cost); ScalarE Lrelu is avoided (ACT_TABLE_LOAD thrash
    + wrong alpha on HW).
"""

import math
from contextlib import ExitStack
from dataclasses import dataclass

import numpy as np
import ml_dtypes

BF16 = ml_dtypes.bfloat16

N_NODES = 40000
N_EDGES = 320000
M1, D1 = 8, 4
M2, D2 = 8, 4
LN_EPS = 1e-5
EQ_EPS = 1e-8
IX1 = np.array([0, 1, 1, 1])
IX2 = np.array([0, 1, 1, 1])

N_CORES = 8
BUCKET_N = 128
NB = 40
SCALE = 32.0 ** -0.5
SHIFT_B = 140.0
CLAMP_A = 1e34
SEL_TH = 1e33

# Precision knobs (validated against numpy sim of the same pipeline):
#   mlp_f32: run the edge MLP matmuls in fp32
#   tmp_f32: run the fU x basis cascade in fp32 (quantize tmp2 to bf16)
OPT = {"mlp_f32": False, "tmp_f32": False, "tmp_gpsimd": False, "tmp_5d": False, "oh_fp8": True}


@dataclass
class Cfg:
    nb: int
    kb: int                      # max chunks per bucket (tile sizing)
    kbp: tuple = None            # per-pair chunk count (len nb//2)

    @property
    def ch(self):
        return self.nb * self.kb

    @property
    def e_pad(self):
        return self.ch * 128


# ---------------------------------------------------------------------------
# Patches: this walrus build allows at most ONE sync wait per instruction.
# ---------------------------------------------------------------------------
_PATCHED = False


def _apply_patches():
    global _PATCHED
    if _PATCHED:
        return
    _PATCHED = True
    import re as _re

    import orjson as _orjson

    import concourse.bass as _bass
    from concourse.tile import TileContext as _TC
    from concourse.vector_clock import ScopedClock as _SC, VectorClock as _VC

    def _drain_and_barrier(self, tick_clock, wait_clock):
        nc = self.nc
        gvals = [int(x) for x in _re.findall(r"\d+", repr(tick_clock.global_clock))]
        nz = [(p, v) for p, v in enumerate(gvals) if v > 0]
        if not nz:
            nc.sync.drain()
        for p, v in nz:
            pvc = _VC()
            pvc.require_at_least(p, v)
            d = nc.sync.drain()
            wait_clock.add_sem_waits(d.ins, _SC({None: pvc}))
        nc.all_engine_barrier()
        assert self.sems is not None
        popped = nc._tile_sem_poison_stack.pop()
        assert popped is self._sem_poison
        nc.clear_and_free_semaphores(list(self.sems.allocated().values()))
        nc.all_engine_barrier()

    def _split_multi_waits(data: bytes) -> bytes:
        j = _orjson.loads(data)
        for fn in j.get("functions", []):
            for bb in fn.get("blocks", []):
                out = []
                for ins in bb.get("instructions", []):
                    si = ins.get("sync_info")
                    ow = (si or {}).get("on_wait") or []
                    if len(ow) > 1:
                        for k, w in enumerate(ow[:-1]):
                            out.append({
                                "debug": ins.get("debug", 0),
                                "engine": ins["engine"],
                                "ins": [],
                                "name": f"{ins['name']}-spw{k}",
                                "opcode": "EventSemaphore",
                                "outs": [],
                                "sync_info": {"on_update": [], "on_wait": [w]},
                            })
                        si["on_wait"] = [ow[-1]]
                    out.append(ins)
                bb["instructions"] = out
        return _orjson.dumps(j)

    _orig_to_json_bytes = _bass.Bass.to_json_bytes

    def _to_json_bytes(self):
        return _split_multi_waits(_orig_to_json_bytes(self))

    _TC._drain_and_barrier = _drain_and_barrier
    _bass.Bass.to_json_bytes = _to_json_bytes


# ---------------------------------------------------------------------------
# Device kernel
# ---------------------------------------------------------------------------
def build_kernel(nc, cfg: Cfg):
    import concourse.bass as bass
    import concourse.mybir as mybir
    from concourse.tile import TileContext

    f32 = mybir.dt.float32
    bf16 = mybir.dt.bfloat16
    fp16 = mybir.dt.float16
    Alu = mybir.AluOpType
    Act = mybir.ActivationFunctionType

    NBk, KB, CH, E_PAD = cfg.nb, cfg.kb, cfg.ch, cfg.e_pad
    HB = KB // 2            # half-bucket chunk count for the big cascade
    mdt = f32 if OPT["mlp_f32"] else fp16
    tdt = f32 if OPT["tmp_f32"] else fp16
    cdt = fp16

    basis_d = nc.dram_tensor("basis_s", (NBk, 128, KB * 64), tdt,
                             kind="ExternalInput")
    fu_d = nc.dram_tensor("fu_s", (NBk, 128, KB * 32), tdt,
                          kind="ExternalInput")
    eft_d = nc.dram_tensor("eft_s", (32, E_PAD), mdt, kind="ExternalInput")
    fp8 = mybir.dt.float8e4
    ohdt = fp8 if OPT.get("oh_fp8") else bf16
    oh_d = nc.dram_tensor("oh_s", (NBk, 128, KB * 128), ohdt,
                          kind="ExternalInput")
    w1t_d = nc.dram_tensor("w1t_s", (32, 64), mdt, kind="ExternalInput")
    b1_d = nc.dram_tensor("b1_s", (64, 1), f32, kind="ExternalInput")
    w2b_d = nc.dram_tensor("w2b_s", (65, 768), mdt, kind="ExternalInput")
    proj_d = nc.dram_tensor("proj_s", (128, 256), bf16, kind="ExternalInput")
    out_d = nc.dram_tensor("out_s", (NBk * 128, 32), bf16,
                           kind="ExternalOutput")

    def vap(base, offset, dims):
        return bass.AP(base.tensor, base.offset + offset, dims)

    with TileContext(nc) as tc:
        with ExitStack() as ctx:
            cpool = ctx.enter_context(tc.tile_pool(name="consts", bufs=1))
            w1t_t = cpool.tile([32, 64], mdt)
            nc.sync.dma_start(out=w1t_t[:], in_=w1t_d.ap())
            b1_t = cpool.tile([64, 1], f32)
            nc.sync.dma_start(out=b1_t[:], in_=b1_d.ap())
            w2b_t = cpool.tile([65, 768], mdt)
            nc.sync.dma_start(out=w2b_t[:], in_=w2b_d.ap())
            proj_t = cpool.tile([128, 256], bf16)
            nc.sync.dma_start(out=proj_t[:], in_=proj_d.ap())
            segS = cpool.tile([128, NBk * 72], f32)
            shiftB = cpool.tile([128, 1], f32)
            nc.vector.memset(shiftB[:], -SHIFT_B)

            bpool = ctx.enter_context(tc.tile_pool(name="edges", bufs=3))
            rpool = ctx.enter_context(tc.tile_pool(name="rw", bufs=2))
            gpool = ctx.enter_context(tc.tile_pool(name="tmp", bufs=2))
            tpool = ctx.enter_context(tc.tile_pool(name="work", bufs=1))
            hpool = ctx.enter_context(
                tc.tile_pool(name="psH", bufs=2, space="PSUM"))
            ppool = ctx.enter_context(
                tc.tile_pool(name="psR", bufs=2, space="PSUM"))
            spool = ctx.enter_context(
                tc.tile_pool(name="psS", bufs=2, space="PSUM"))

            # persistent h65 pair with the ones-row set once
            h65s = [cpool.tile([65, 128], mdt, name=f"h65p{k}")
                    for k in range(2)]
            for t in h65s:
                nc.gpsimd.memset(t[64:65, :], 1.0)

            def load_bucket(b, kb):
                basis_b = bpool.tile([128, KB * 64], tdt, tag="basisb")
                nc.sync.dma_start(
                    out=vap(basis_b[:], 0, [[KB * 64, 128], [1, kb * 64]]),
                    in_=vap(basis_d.ap(), b * 128 * KB * 64,
                            [[KB * 64, 128], [1, kb * 64]]))
                fu_b = bpool.tile([128, KB * 32], tdt, tag="fub")
                nc.sync.dma_start(
                    out=vap(fu_b[:], 0, [[KB * 32, 128], [1, kb * 32]]),
                    in_=vap(fu_d.ap(), b * 128 * KB * 32,
                            [[KB * 32, 128], [1, kb * 32]]))
                eft_b = bpool.tile([32, KB * 128], mdt, tag="eftb")
                nc.sync.dma_start(
                    out=vap(eft_b[:], 0, [[KB * 128, 32], [1, kb * 128]]),
                    in_=vap(eft_d.ap(), b * KB * 128,
                            [[E_PAD, 32], [1, kb * 128]]))
                oh_b = bpool.tile([128, KB * 128], ohdt, tag="ohb")
                nc.sync.dma_start(
                    out=vap(oh_b[:], 0, [[KB * 128, 128], [1, kb * 128]]),
                    in_=vap(oh_d.ap(), b * 128 * KB * 128,
                            [[KB * 128, 128], [1, kb * 128]]))
                return basis_b, fu_b, eft_b, oh_b

            def mlp_bucket(b, eft_b, kb):
                rwb = rpool.tile([128, KB * 768], cdt, tag="rwb")
                for i in range(kb):
                    hps = hpool.tile([64, 128], f32, tag="hps")
                    nc.tensor.matmul(out=hps[:], lhsT=w1t_t[:],
                                     rhs=eft_b[:, i * 128:(i + 1) * 128],
                                     start=True, stop=True)
                    h65 = h65s[i % 2]
                    nc.scalar.activation(h65[0:64, :], hps[:], Act.Relu,
                                         bias=b1_t[:, 0:1])
                    rwp = ppool.tile([128, 768], f32, tag="rwp")
                    nc.tensor.matmul(out=rwp[:, 0:512], lhsT=h65[:],
                                     rhs=w2b_t[:, 0:512], start=True,
                                     stop=True)
                    nc.tensor.matmul(out=rwp[:, 512:768], lhsT=h65[:],
                                     rhs=w2b_t[:, 512:768], start=True,
                                     stop=True)
                    nc.scalar.activation(rwb[:, i * 768:(i + 1) * 768],
                                         rwp[:], Act.Copy)
                return rwb

            def tmp_bucket(b, basis_b, fu_b, kb):
                """fU x basis products + d1-sum -> tmp2 in (d2, j) layout.

                The d1-sum is a 2-level halving tree instead of a
                tensor_reduce: reduce only has a 1x uop (FD cycles), while
                the first tree level runs at 2x (pairs (d1,d1+2) keep both
                operands stride-1 / 4B-aligned). L2 pairs are stride-2 ->
                1x, but its FD is only a quarter of the reduce's.
                """
                tmp2 = gpool.tile([128, KB * 128], cdt, tag="tmp2")
                tmp2a = tmp2[:]
                ptb = gpool.tile([128, KB * 512], tdt, tag="ptb")
                ptba = ptb[:]
                # ptb layout per edge: [pd(16):32][m(8):4][d1(4):1]; the host
                # permutes basis cols to pd=(d2,p) and w2 rows to j=(p,m), so
                # the tree's natural group order 8*pd+m lands tmp2 directly
                # in P0's [d2:32][(p,m):1] layout -- every tree op then has a
                # unit-stride output (strided DVE writes cost ~4.3 cyc/elem).
                nc.vector.tensor_tensor(
                    vap(ptba, 0,
                        [[KB * 512, 128], [512, kb], [32, 16], [4, 8],
                         [1, 4]]),
                    vap(fu_b[:], 0,
                        [[KB * 32, 128], [32, kb], [0, 16], [4, 8],
                         [1, 4]]),
                    vap(basis_b[:], 0,
                        [[KB * 64, 128], [64, kb], [4, 16], [0, 8],
                         [1, 4]]),
                    Alu.mult)
                h1 = tpool.tile([128, KB * 256], cdt, tag="h1")
                h1a = h1[:]
                with nc.allow_low_precision(reason="fp16 tmp"):
                    nc.vector.tensor_tensor(
                        vap(h1a, 0,
                            [[KB * 256, 128], [256, kb], [2, 128], [1, 2]]),
                        vap(ptba, 0,
                            [[KB * 512, 128], [512, kb], [4, 128], [1, 2]]),
                        vap(ptba, 2,
                            [[KB * 512, 128], [512, kb], [4, 128], [1, 2]]),
                        Alu.add)
                    nc.vector.tensor_tensor(
                        vap(tmp2a, 0,
                            [[KB * 128, 128], [128, kb], [1, 128]]),
                        vap(h1a, 0,
                            [[KB * 256, 128], [256, kb], [2, 128]]),
                        vap(h1a, 1,
                            [[KB * 256, 128], [256, kb], [2, 128]]),
                        Alu.add)
                return tmp2

            # Pair-level tiles: the cascade fills half-slots per bucket and
            # the score/payload ops run once per 2 buckets with doubled FD,
            # halving their fixed per-instruction cost (~151 DVE cycles).
            KB2 = 2 * KB
            conv2 = tpool.tile([128, KB2 * 64], cdt, name="conv2")
            convV2 = tpool.tile([128, KB2 * 32], bf16, name="convV2")
            Y2 = tpool.tile([128, KB2 * 72], bf16, name="Y2")
            ex82 = tpool.tile([128, KB2 * 64], bf16, name="ex82")
            ps2 = tpool.tile([128, KB2 * 32], cdt, name="ps2")
            sc42 = tpool.tile([128, KB2 * 4], f32, name="sc42")
            scl2 = tpool.tile([128, KB2 * 4], f32, name="scl2")

            def conv_bucket(b, rwb, tmp2, kb):
                rwba = rwb[:]
                tmp2a = tmp2[:]
                half = (b & 1) * kb
                P0 = tpool.tile([128, KB * 3072], cdt, tag="P0")
                T1 = tpool.tile([128, KB * 1536], cdt, tag="T1")
                T2 = tpool.tile([128, KB * 768], cdt, tag="T2")
                T3 = tpool.tile([128, KB * 384], cdt, tag="T3")
                T4 = tpool.tile([128, KB * 192], cdt, tag="T4")
                nc.vector.tensor_tensor(
                    vap(P0[:], 0,
                        [[KB * 3072, 128], [3072, kb], [128, 24],
                         [32, 4], [1, 32]]),
                    vap(rwba, 0,
                        [[KB * 768, 128], [768, kb], [32, 24], [0, 4],
                         [1, 32]]),
                    vap(tmp2a, 0,
                        [[KB * 128, 128], [128, kb], [0, 24], [32, 4],
                         [1, 32]]),
                    Alu.mult)
                cur, w = P0[:], 32
                nxts = {16: T1, 8: T2, 4: T3, 2: T4}
                while w > 2:
                    w2 = w // 2
                    nxt = nxts[w2]
                    nc.vector.tensor_tensor(
                        vap(nxt[:], 0,
                            [[KB * 96 * w2, 128], [96 * w2, kb],
                             [4 * w2, 24], [w2, 4], [1, w2]]),
                        vap(cur, 0,
                            [[KB * 96 * w, 128], [96 * w, kb],
                             [4 * w, 24], [w, 4], [1, w2]]),
                        vap(cur, w2,
                            [[KB * 96 * w, 128], [96 * w, kb],
                             [4 * w, 24], [w, 4], [1, w2]]),
                        Alu.add)
                    cur, w = nxt[:], w2
                nc.vector.tensor_tensor(
                    vap(conv2[:], half * 64,
                        [[KB2 * 64, 128], [64, kb], [4, 16], [1, 4]]),
                    vap(cur, 0,
                        [[KB * 192, 128], [192, kb], [8, 16], [2, 4]]),
                    vap(cur, 1,
                        [[KB * 192, 128], [192, kb], [8, 16], [2, 4]]),
                    Alu.add)
                nc.vector.tensor_tensor(
                    vap(convV2[:], half * 32,
                        [[KB2 * 32, 128], [32, kb], [4, 8], [1, 4]]),
                    vap(cur, 128,
                        [[KB * 192, 128], [192, kb], [8, 8], [2, 4]]),
                    vap(cur, 129,
                        [[KB * 192, 128], [192, kb], [8, 8], [2, 4]]),
                    Alu.add)

            def scores_pair(b0, oh_pair, kb):
                """Scores + dual-exp softmax + payload for buckets b0,b0+1."""
                kb2 = 2 * kb
                Ya = Y2[:]
                nc.vector.tensor_tensor(
                    vap(ps2[:], 0,
                        [[KB2 * 32, 128], [1, kb2 * 32]]),
                    vap(conv2[:], 0,
                        [[KB2 * 64, 128], [64, kb2], [1, 32]]),
                    vap(conv2[:], 32,
                        [[KB2 * 64, 128], [64, kb2], [1, 32]]),
                    Alu.mult)
                nc.vector.tensor_reduce(
                    vap(sc42[:], 0,
                        [[KB2 * 4, 128], [1, kb2 * 4]]),
                    vap(ps2[:], 0,
                        [[KB2 * 32, 128], [8, kb2 * 4], [1, 8]]),
                    mybir.AxisListType.X, Alu.add)
                # SCALE is folded into the k-rows of w2 on the host.
                # (ScalarE Lrelu was tried here: it thrashes the activation
                # table -- 1.3us ACT_TABLE_LOAD per swap vs Exp -- and its
                # alpha lowering looked wrong on HW. Keep leaky on DVE.)
                sc4a = vap(sc42[:], 0, [[KB2 * 4, 128], [1, kb2 * 4]])
                scla = vap(scl2[:], 0, [[KB2 * 4, 128], [1, kb2 * 4]])
                nc.vector.scalar_tensor_tensor(
                    scla, sc4a, 0.2, sc4a, Alu.mult, Alu.max)
                nc.scalar.activation(
                    vap(Ya, 32, [[KB2 * 72, 128], [72, kb2], [1, 4]]),
                    scla, Act.Exp)
                nc.vector.tensor_scalar(
                    vap(Ya, 32, [[KB2 * 72, 128], [72, kb2], [1, 4]]),
                    vap(Ya, 32, [[KB2 * 72, 128], [72, kb2], [1, 4]]),
                    CLAMP_A, None, Alu.min)
                nc.scalar.activation(
                    vap(Ya, 68, [[KB2 * 72, 128], [72, kb2], [1, 4]]),
                    scla, Act.Exp, bias=shiftB[:, 0:1])
                # Pre-expand the 4 per-head exp values to 8-wide on the
                # ScalarE so the payload multiply keeps stride-1 operands
                # (2x DVE mode); a [0,8]-broadcast operand would force 1x.
                nc.scalar.activation(
                    vap(ex82[:], 0,
                        [[KB2 * 64, 128], [64, kb2], [32, 2], [8, 4],
                         [1, 8]]),
                    vap(Ya, 32,
                        [[KB2 * 72, 128], [72, kb2], [36, 2], [1, 4],
                         [0, 8]]),
                    Act.Copy)
                nc.vector.tensor_tensor(
                    vap(Ya, 0,
                        [[KB2 * 72, 128], [72, kb2], [36, 2], [8, 4],
                         [1, 8]]),
                    vap(convV2[:], 0,
                        [[KB2 * 32, 128], [32, kb2], [0, 2], [8, 4],
                         [1, 8]]),
                    vap(ex82[:], 0,
                        [[KB2 * 64, 128], [64, kb2], [32, 2], [8, 4],
                         [1, 8]]),
                    Alu.mult)

                # ---- segment matmuls (one-hot from host), per bucket
                for h in range(2):
                    seg = spool.tile([128, 72], f32, tag="seg")
                    for i in range(kb):
                        nc.tensor.matmul(
                            out=seg[:],
                            lhsT=oh_pair[h][:, i * 128:(i + 1) * 128],
                            rhs=Y2[:, (h * kb + i) * 72:
                                   (h * kb + i + 1) * 72],
                            start=(i == 0), stop=(i == kb - 1))
                    nc.scalar.activation(
                        segS[:, (b0 + h) * 72:(b0 + h + 1) * 72], seg[:],
                        Act.Copy)

            # ---- software pipeline: MLP + tmp run one bucket ahead
            KBP = cfg.kbp if cfg.kbp is not None else (KB,) * (NBk // 2)
            prev = None
            oh_even = None
            for b in range(NBk):
                kb = KBP[b // 2]
                basis_b, fu_b, eft_b, oh_b = load_bucket(b, kb)
                rwb = mlp_bucket(b, eft_b, kb)
                tmp2 = tmp_bucket(b, basis_b, fu_b, kb)
                if prev is not None:
                    pb, prwb, ptmp2, poh, pkb = prev
                    conv_bucket(pb, prwb, ptmp2, pkb)
                    if pb & 1:
                        scores_pair(pb - 1, (oh_even, poh), pkb)
                    else:
                        oh_even = poh
                prev = (b, rwb, tmp2, oh_b, kb)
            pb, prwb, ptmp2, poh, pkb = prev
            conv_bucket(pb, prwb, ptmp2, pkb)
            scores_pair(pb - 1, (oh_even, poh), pkb)

            # ======== Phase 3: select pass, divide, project, store ========
            segA = segS[:]
            rdA = cpool.tile([128, NBk * 4], f32)
            nc.vector.tensor_scalar(
                rdA[:], vap(segA, 32, [[NBk * 72, 128], [72, NBk], [1, 4]]),
                1e-30, None, Alu.add)
            nc.vector.reciprocal(rdA[:], rdA[:])
            rdB = cpool.tile([128, NBk * 4], f32)
            nc.vector.tensor_scalar(
                rdB[:], vap(segA, 68, [[NBk * 72, 128], [72, NBk], [1, 4]]),
                1e-30, None, Alu.add)
            nc.vector.reciprocal(rdB[:], rdB[:])
            msk = cpool.tile([128, NBk * 4], f32)
            nc.vector.tensor_scalar(
                msk[:], vap(segA, 32, [[NBk * 72, 128], [72, NBk], [1, 4]]),
                SEL_TH, None, Alu.is_lt)
            oA = cpool.tile([128, NBk * 32], f32)
            nc.vector.tensor_tensor(
                vap(oA[:], 0, [[NBk * 32, 128], [32, NBk], [8, 4], [1, 8]]),
                vap(segA, 0, [[NBk * 72, 128], [72, NBk], [8, 4], [1, 8]]),
                vap(rdA[:], 0, [[NBk * 4, 128], [4, NBk], [1, 4], [0, 8]]),
                Alu.mult)
            oB = cpool.tile([128, NBk * 32], f32)
            nc.vector.tensor_tensor(
                vap(oB[:], 0, [[NBk * 32, 128], [32, NBk], [8, 4], [1, 8]]),
                vap(segA, 36, [[NBk * 72, 128], [72, NBk], [8, 4], [1, 8]]),
                vap(rdB[:], 0, [[NBk * 4, 128], [4, NBk], [1, 4], [0, 8]]),
                Alu.mult)
            osc = cpool.tile([128, NBk * 32], bf16)
            osca = osc[:]
            nc.vector.tensor_tensor(oA[:], oA[:], oB[:], Alu.subtract)
            nc.vector.tensor_tensor(
                vap(oA[:], 0, [[NBk * 32, 128], [32, NBk], [8, 4], [1, 8]]),
                vap(oA[:], 0, [[NBk * 32, 128], [32, NBk], [8, 4], [1, 8]]),
                vap(msk[:], 0, [[NBk * 4, 128], [4, NBk], [1, 4], [0, 8]]),
                Alu.mult)
            nc.vector.tensor_tensor(osc[:], oA[:], oB[:], Alu.add)
            res = cpool.tile([128, NBk * 32], bf16)
            resa = res[:]
            scr = cpool.tile([128, NBk * 32], bf16)
            scra = scr[:]
            for mp in range(8):
                tgt = resa if mp == 0 else scra
                nc.vector.tensor_tensor(
                    vap(tgt, 0, [[NBk * 32, 128], [32, NBk], [4, 8], [1, 4]]),
                    vap(osca, mp * 4,
                        [[NBk * 32, 128], [32, NBk], [0, 8], [1, 4]]),
                    vap(proj_t[:], mp * 32,
                        [[256, 128], [0, NBk], [4, 8], [1, 4]]),
                    Alu.mult)
                if mp > 0:
                    nc.vector.tensor_tensor(resa, resa, scra, Alu.add)
            nc.sync.dma_start(
                out=vap(out_d.ap(), 0, [[32, 128], [4096, NBk], [1, 32]]),
                in_=res[:])
    return nc


# ---------------------------------------------------------------------------
# Host-side prep
# ---------------------------------------------------------------------------
def _host_ln(features, ln_w, ln_b):
    f32 = np.float32
    feats = features.reshape(-1, M1, D1).astype(f32)
    onehot = np.eye(2, dtype=f32)[IX1]
    norms = np.sqrt((feats ** 2) @ onehot)
    x = norms.reshape(-1, 2, 8)
    mu = x.mean(-1, keepdims=True, dtype=f32).astype(f32)
    var = ((x - mu) ** 2).mean(-1, keepdims=True, dtype=f32).astype(f32)
    ln = (x - mu) / np.sqrt(var + LN_EPS) * ln_w + ln_b
    ln = np.maximum(ln, 0).astype(f32).reshape(-1, M1, 2)
    return (feats * (ln / (norms + EQ_EPS))[:, :, IX1]).astype(f32)


def _prep(inputs, cfg: Cfg = None):
    mdt = np.float32 if OPT["mlp_f32"] else np.float16
    tdt = np.float32 if OPT["tmp_f32"] else np.float16

    src = np.asarray(inputs["src"]).astype(np.int64)
    dst = np.asarray(inputs["dst"]).astype(np.int64)
    n_nodes = np.asarray(inputs["features"]).shape[0]
    # basis: (E, d1, pd) -> per-edge (pd, d1) so products are stride-1.
    # pd columns permuted from k=(p,d2) to pd_hat=(d2,p) so the device
    # tree's group order 8*pd_hat+m equals 32*d2 + (8p+m) = P0's layout.
    basis = np.asarray(inputs["basis"], np.float32).reshape(-1, 4, 16)
    pd_perm = np.array([p * 4 + d2 for d2 in range(4) for p in range(4)])
    basis = basis[:, :, pd_perm]
    basis = np.ascontiguousarray(basis.transpose(0, 2, 1)).reshape(-1, 64)
    ef = np.asarray(inputs["edge_feats"], np.float32)

    nb_l = cfg.nb if cfg is not None else NB
    nb_g = N_CORES * nb_l
    nodes_pad = nb_g * BUCKET_N

    deg = np.bincount(dst, minlength=nodes_pad)
    order = np.argsort(-deg, kind="stable")
    assign = np.empty(nodes_pad, dtype=np.int64)
    pos = np.empty(nodes_pad, dtype=np.int64)
    # Phase 1: global degree-snake, only to balance edges across CORES.
    for r in range(BUCKET_N):
        sl = order[r * nb_g:(r + 1) * nb_g]
        buckets = np.arange(nb_g) if r % 2 == 0 else np.arange(nb_g)[::-1]
        assign[sl] = buckets
        pos[sl] = r
    # Phase 2: per-core repack -- the last LOW_N buckets of each core are
    # packed light (<= 7*128 edges) so their pairs run with one fewer
    # 128-edge chunk; the rest stay <= 8*128.  A sliding window over the
    # degree-sorted nodes picks a 512-node subset summing ~4*890 for the
    # light buckets; each group is then deficit-snaked into its buckets.
    LOW_N, LOW_CAP = 4, 7 * BUCKET_N
    core_of = assign // nb_l
    for core in range(N_CORES):
        idx = np.where(core_of == core)[0]
        idx = idx[np.argsort(-deg[idx], kind="stable")]
        nlow = LOW_N * BUCKET_N
        low_target = LOW_N * (LOW_CAP - 6)
        csum = np.concatenate([[0], np.cumsum(deg[idx])])
        wsums = csum[nlow:] - csum[:-nlow]
        w0 = int(np.argmin(np.abs(wsums - low_target)))
        low_idx = idx[w0:w0 + nlow]
        hi_idx = np.concatenate([idx[:w0], idx[w0 + nlow:]])
        nhi = nb_l - LOW_N
        tgt_hi = np.full(nhi, float(deg[hi_idx].sum()) / nhi)
        tgt_lo = np.full(LOW_N, float(deg[low_idx].sum()) / LOW_N)
        lb_hi = np.zeros(nhi)
        lb_lo = np.zeros(LOW_N)
        for r in range(BUCKET_N):
            grp = hi_idx[r * nhi:(r + 1) * nhi]
            bo = np.argsort(-(tgt_hi - lb_hi), kind="stable")
            assign[grp] = core * nb_l + bo
            pos[grp] = r
            np.add.at(lb_hi, bo, deg[grp].astype(np.float64))
            grp2 = low_idx[r * LOW_N:(r + 1) * LOW_N]
            bo2 = np.argsort(-(tgt_lo - lb_lo), kind="stable")
            assign[grp2] = core * nb_l + nhi + bo2
            pos[grp2] = r
            np.add.at(lb_lo, bo2, deg[grp2].astype(np.float64))
    loads = np.zeros(nb_g, dtype=np.int64)
    np.add.at(loads, assign[dst], 1)
    kb = int(math.ceil(loads.max() / 128.0))
    kb += kb & 1                       # pair split needs even max kb
    # Per-pair chunk counts: max over the two buckets and all 8 cores.
    kb_bkt = np.ceil(loads.reshape(N_CORES, nb_l) / 128.0).astype(int)
    kb_bkt = kb_bkt.max(axis=0)        # SPMD: one NEFF for all cores
    kbp = tuple(int(max(kb_bkt[2 * p], kb_bkt[2 * p + 1]))
                for p in range(nb_l // 2))
    if cfg is None:
        cfg = Cfg(nb=nb_l, kb=kb, kbp=kbp)
    assert kb <= cfg.kb, f"kb={kb} exceeds cfg.kb={cfg.kb}"

    f = _host_ln(np.asarray(inputs["features"], np.float32),
                 np.asarray(inputs["ln_w"], np.float32),
                 np.asarray(inputs["ln_b"], np.float32))
    fU_all = f[src].reshape(-1, 32)

    eb = assign[dst]
    eorder = np.argsort(eb, kind="stable")
    bstart = np.searchsorted(eb[eorder], np.arange(nb_g + 1))

    E_PAD, CH, KB = cfg.e_pad, cfg.ch, cfg.kb
    arange128 = np.arange(128, dtype=np.int64)
    in_maps = []
    for core in range(N_CORES):
        basis_s = np.zeros((E_PAD, 64), np.float32)
        fu_s = np.zeros((E_PAD, 32), np.float32)
        eft_s = np.zeros((32, E_PAD), np.float32)
        dstrel_s = np.full((E_PAD,), -1, np.int64)
        for lb in range(cfg.nb):
            gb = core * cfg.nb + lb
            eidx = eorder[bstart[gb]:bstart[gb + 1]]
            n = len(eidx)
            assert n <= cfg.kbp[lb // 2] * 128 if cfg.kbp else n <= KB * 128
            o = lb * KB * 128
            basis_s[o:o + n] = basis[eidx]
            fu_s[o:o + n] = fU_all[eidx]
            eft_s[:, o:o + n] = ef[eidx].T
            dstrel_s[o:o + n] = pos[dst[eidx]]
        # bucket-block layouts: (NB, 128, KB*w); edge (chunk i, part p)
        basis_bb = (basis_s.reshape(cfg.nb, KB, 128, 64)
                    .transpose(0, 2, 1, 3).reshape(cfg.nb, 128, KB * 64))
        fu_bb = (fu_s.reshape(cfg.nb, KB, 128, 32)
                 .transpose(0, 2, 1, 3).reshape(cfg.nb, 128, KB * 32))
        # one-hot: (NB, KB, 128e) -> (NB, 128e, KB*128n)
        oh_np = (ml_dtypes.float8_e4m3 if OPT.get("oh_fp8") else BF16)
        oh = (dstrel_s.reshape(cfg.nb, KB, 128)[..., None] ==
              arange128).astype(oh_np)
        oh_bb = np.ascontiguousarray(
            oh.transpose(0, 2, 1, 3)).reshape(cfg.nb, 128, KB * 128)
        in_maps.append({
            "basis_s": np.ascontiguousarray(basis_bb).astype(tdt),
            "fu_s": np.ascontiguousarray(fu_bb).astype(tdt),
            "eft_s": eft_s.astype(mdt),
            "oh_s": oh_bb,
        })

    w1 = np.asarray(inputs["w1"], np.float32)
    b1 = np.asarray(inputs["b1"], np.float32).reshape(64, 1)
    w2 = np.asarray(inputs["w2"], np.float32)
    b2 = np.asarray(inputs["b2"], np.float32)
    w2s = w2.copy()
    b2s = b2.copy()
    w2s[0:256, :] *= SCALE          # k-rows: fold the attention scale in
    b2s[0:256] *= SCALE
    # rw column order (i, m, p) -> (i, p, m): matches tmp2's (p, m) inner
    # layout so P0's stride-1 inner dim pairs rw and tmp consistently.
    jp = np.array([p * 8 + m for m in range(8) for p in range(4)])
    rperm = (np.arange(24)[:, None] * 32 + jp[None, :]).ravel()
    inv = np.empty_like(rperm)
    inv[rperm] = np.arange(768)
    w2s = w2s[inv]
    b2s = b2s[inv]
    w2b = np.concatenate([w2s.T, b2s[None, :]], axis=0).astype(np.float32)
    projw = np.asarray(inputs["proj_w"], np.float32)
    ptbl_flat = np.zeros((256,), np.float32)
    for mpi in range(8):
        for m in range(8):
            for d in range(4):
                ptbl_flat[mpi * 32 + m * 4 + d] = projw[IX2[d] * 8 + m, mpi]
    ptbl = np.broadcast_to(ptbl_flat, (128, 256)).astype(BF16)
    for im in in_maps:
        im.update({
            "w1t_s": np.ascontiguousarray(w1.T).astype(mdt),
            "b1_s": b1,
            "w2b_s": w2b.astype(mdt),
            "proj_s": ptbl,
        })
    meta = {"assign": assign, "pos": pos, "n_nodes": n_nodes}
    return in_maps, meta, cfg


def _unshard(results, meta):
    out_cat = np.concatenate([r["out_s"] for r in results], axis=0)
    assign, pos, n = meta["assign"], meta["pos"], meta["n_nodes"]
    rows = assign[:n] * 128 + pos[:n]
    return out_cat[rows].reshape(n, M2, D2)


def _run(inputs, trace=False):
    _apply_patches()
    import concourse.bass as bass
    from concourse.bass_utils import run_bass_kernel_spmd

    in_maps, meta, cfg = _prep(inputs)
    nc = bass.Bass("TRN2", target_bir_lowering=False)
    build_kernel(nc, cfg)
    r = run_bass_kernel_spmd(nc, in_maps, core_ids=list(range(N_CORES)),
                             trace=trace)
    out = _unshard(r.results, meta)
    return out, r


def kernel(**inputs) -> np.ndarray:
    out, _ = _run(inputs, trace=False)
    return out.astype(np.float32)



# revision 30
# speedup vs baseline: 1.0198x; 1.0054x over previous
"""Trainium2 Bass kernel for nn_EquivariantTransformerBlock.

Strategy (8 NeuronCores, no collectives, no indirect DMA):
  - Host assigns each node to one of 320 buckets of 128 nodes (degree-
    balanced snake packing); every edge goes to the core owning its dst
    bucket, so all segment sums are core-local.
  - Host computes the (tiny) equivariant LayerNorm, gathers f[src], and
    precomputes per-chunk one-hot matrices (fp8) for the segment sums.
  - Device pipeline, bucket (KB x 128 edges) at a time, software-
    pipelined one bucket ahead (PE MLP + ScalarE casts + DMA run under
    the DVE cascade of the previous bucket):
      * edge MLP on the TensorE in fp16; PSUM->SBUF casts on ScalarE,
      * fU x basis and rw x tmp contractions as fp16 broadcast-product
        + halving-tree ops on the VectorE (fp16 keeps the DVE 2x mode
        with 4x finer rounding than bf16; scores need that precision),
      * attention scores with the 1/sqrt(h) scale folded into the
        k-rows of w2 on the host,
      * dual-exp softmax (shift-free: clamped exp(s) / exp(s-140) with
        per-node select in the epilogue) with bf16 payload range,
      * segment sums as one-hot matmuls accumulated in PSUM per bucket.

DVE scheduling rules this kernel is tuned around (hardware-measured):
  - tensor_tensor runs 2x (2 elem/cyc/lane) only with 16-bit dtypes and
    unit-stride, 4B-aligned innermost dims on ALL operands; ~151 cycles
    fixed cost per instruction.
  - tensor_reduce has only a 1x uop -> the d1-sum over the fU x basis
    products is a 2-level halving tree instead (L1 at 2x); the host
    permutes basis cols to pd=(d2,p) and w2 rows to j=(p,m) so every
    tree level writes unit-stride (strided writes cost ~4.3 cyc/elem).
  - the per-head exp values are pre-expanded 8-wide on the ScalarE so
    the payload multiply keeps 2x (a [0,8]-broadcast operand forces 1x).
  - score/softmax/payload ops run once per bucket PAIR (doubled FD,
    halved fixed cost); ScalarE Lrelu is avoided (ACT_TABLE_LOAD thrash
    + wrong alpha on HW).
  - mixed-kb packing: the last 2 bucket pairs of each core are packed
    light (<= 896 edges/bucket via a sliding-window node subset) and run
    with kb=7 chunks instead of 8, trimming 4/320 of the edge-padding
    work; all per-bucket APs/loops take kb per pair (SPMD: kbp is the
    max over cores).
"""

import math
from contextlib import ExitStack
from dataclasses import dataclass

import numpy as np
import ml_dtypes

BF16 = ml_dtypes.bfloat16

N_NODES = 40000
N_EDGES = 320000
M1, D1 = 8, 4
M2, D2 = 8, 4
LN_EPS = 1e-5
EQ_EPS = 1e-8
IX1 = np.array([0, 1, 1, 1])
IX2 = np.array([0, 1, 1, 1])

N_CORES = 8
BUCKET_N = 128
NB = 40
SCALE = 32.0 ** -0.5
SHIFT_B = 140.0
CLAMP_A = 1e34
SEL_TH = 1e33

# Precision knobs (validated against numpy sim of the same pipeline):
#   mlp_f32: run the edge MLP matmuls in fp32
#   tmp_f32: run the fU x basis cascade in fp32 (quantize tmp2 to bf16)
OPT = {"mlp_f32": False, "tmp_f32": False, "tmp_gpsimd": False, "tmp_5d": False, "oh_fp8": True}


@dataclass
class Cfg:
    nb: int
    kb: int                      # max chunks per bucket (tile sizing)
    kbp: tuple = None            # per-pair chunk count (len nb//2)

    @property
    def ch(self):
        return self.nb * self.kb

    @property
    def e_pad(self):
        return self.ch * 128


# ---------------------------------------------------------------------------
# Patches: this walrus build allows at most ONE sync wait per instruction.
# ---------------------------------------------------------------------------
_PATCHED = False


def _apply_patches():
    global _PATCHED
    if _PATCHED:
        return
    _PATCHED = True
    import re as _re

    import orjson as _orjson

    import concourse.bass as _bass
    from concourse.tile import TileContext as _TC
    from concourse.vector_clock import ScopedClock as _SC, VectorClock as _VC

    def _drain_and_barrier(self, tick_clock, wait_clock):
        nc = self.nc
        gvals = [int(x) for x in _re.findall(r"\d+", repr(tick_clock.global_clock))]
        nz = [(p, v) for p, v in enumerate(gvals) if v > 0]
        if not nz:
            nc.sync.drain()
        for p, v in nz:
            pvc = _VC()
            pvc.require_at_least(p, v)
            d = nc.sync.drain()
            wait_clock.add_sem_waits(d.ins, _SC({None: pvc}))
        nc.all_engine_barrier()
        assert self.sems is not None
        popped = nc._tile_sem_poison_stack.pop()
        assert popped is self._sem_poison
        nc.clear_and_free_semaphores(list(self.sems.allocated().values()))
        nc.all_engine_barrier()

    def _split_multi_waits(data: bytes) -> bytes:
        j = _orjson.loads(data)
        for fn in j.get("functions", []):
            for bb in fn.get("blocks", []):
                out = []
                for ins in bb.get("instructions", []):
                    si = ins.get("sync_info")
                    ow = (si or {}).get("on_wait") or []
                    if len(ow) > 1:
                        for k, w in enumerate(ow[:-1]):
                            out.append({
                                "debug": ins.get("debug", 0),
                                "engine": ins["engine"],
                                "ins": [],
                                "name": f"{ins['name']}-spw{k}",
                                "opcode": "EventSemaphore",
                                "outs": [],
                                "sync_info": {"on_update": [], "on_wait": [w]},
                            })
                        si["on_wait"] = [ow[-1]]
                    out.append(ins)
                bb["instructions"] = out
        return _orjson.dumps(j)

    _orig_to_json_bytes = _bass.Bass.to_json_bytes

    def _to_json_bytes(self):
        return _split_multi_waits(_orig_to_json_bytes(self))

    _TC._drain_and_barrier = _drain_and_barrier
    _bass.Bass.to_json_bytes = _to_json_bytes


# ---------------------------------------------------------------------------
# Device kernel
# ---------------------------------------------------------------------------
def build_kernel(nc, cfg: Cfg):
    import concourse.bass as bass
    import concourse.mybir as mybir
    from concourse.tile import TileContext

    f32 = mybir.dt.float32
    bf16 = mybir.dt.bfloat16
    fp16 = mybir.dt.float16
    Alu = mybir.AluOpType
    Act = mybir.ActivationFunctionType

    NBk, KB, CH, E_PAD = cfg.nb, cfg.kb, cfg.ch, cfg.e_pad
    HB = KB // 2            # half-bucket chunk count for the big cascade
    mdt = f32 if OPT["mlp_f32"] else fp16
    tdt = f32 if OPT["tmp_f32"] else fp16
    cdt = fp16

    basis_d = nc.dram_tensor("basis_s", (NBk, 128, KB * 64), tdt,
                             kind="ExternalInput")
    fu_d = nc.dram_tensor("fu_s", (NBk, 128, KB * 32), tdt,
                          kind="ExternalInput")
    eft_d = nc.dram_tensor("eft_s", (32, E_PAD), mdt, kind="ExternalInput")
    fp8 = mybir.dt.float8e4
    ohdt = fp8 if OPT.get("oh_fp8") else bf16
    oh_d = nc.dram_tensor("oh_s", (NBk, 128, KB * 128), ohdt,
                          kind="ExternalInput")
    w1t_d = nc.dram_tensor("w1t_s", (32, 64), mdt, kind="ExternalInput")
    b1_d = nc.dram_tensor("b1_s", (64, 1), f32, kind="ExternalInput")
    w2b_d = nc.dram_tensor("w2b_s", (65, 768), mdt, kind="ExternalInput")
    proj_d = nc.dram_tensor("proj_s", (128, 256), bf16, kind="ExternalInput")
    out_d = nc.dram_tensor("out_s", (NBk * 128, 32), bf16,
                           kind="ExternalOutput")

    def vap(base, offset, dims):
        return bass.AP(base.tensor, base.offset + offset, dims)

    with TileContext(nc) as tc:
        with ExitStack() as ctx:
            cpool = ctx.enter_context(tc.tile_pool(name="consts", bufs=1))
            w1t_t = cpool.tile([32, 64], mdt)
            nc.sync.dma_start(out=w1t_t[:], in_=w1t_d.ap())
            b1_t = cpool.tile([64, 1], f32)
            nc.sync.dma_start(out=b1_t[:], in_=b1_d.ap())
            w2b_t = cpool.tile([65, 768], mdt)
            nc.sync.dma_start(out=w2b_t[:], in_=w2b_d.ap())
            proj_t = cpool.tile([128, 256], bf16)
            nc.sync.dma_start(out=proj_t[:], in_=proj_d.ap())
            segS = cpool.tile([128, NBk * 72], f32)
            shiftB = cpool.tile([128, 1], f32)
            nc.vector.memset(shiftB[:], -SHIFT_B)

            bpool = ctx.enter_context(tc.tile_pool(name="edges", bufs=3))
            rpool = ctx.enter_context(tc.tile_pool(name="rw", bufs=2))
            gpool = ctx.enter_context(tc.tile_pool(name="tmp", bufs=2))
            tpool = ctx.enter_context(tc.tile_pool(name="work", bufs=1))
            hpool = ctx.enter_context(
                tc.tile_pool(name="psH", bufs=2, space="PSUM"))
            ppool = ctx.enter_context(
                tc.tile_pool(name="psR", bufs=2, space="PSUM"))
            spool = ctx.enter_context(
                tc.tile_pool(name="psS", bufs=2, space="PSUM"))

            # persistent h65 pair with the ones-row set once
            h65s = [cpool.tile([65, 128], mdt, name=f"h65p{k}")
                    for k in range(2)]
            for t in h65s:
                nc.gpsimd.memset(t[64:65, :], 1.0)

            def load_bucket(b, kb):
                basis_b = bpool.tile([128, KB * 64], tdt, tag="basisb")
                nc.sync.dma_start(
                    out=vap(basis_b[:], 0, [[KB * 64, 128], [1, kb * 64]]),
                    in_=vap(basis_d.ap(), b * 128 * KB * 64,
                            [[KB * 64, 128], [1, kb * 64]]))
                fu_b = bpool.tile([128, KB * 32], tdt, tag="fub")
                nc.sync.dma_start(
                    out=vap(fu_b[:], 0, [[KB * 32, 128], [1, kb * 32]]),
                    in_=vap(fu_d.ap(), b * 128 * KB * 32,
                            [[KB * 32, 128], [1, kb * 32]]))
                eft_b = bpool.tile([32, KB * 128], mdt, tag="eftb")
                nc.sync.dma_start(
                    out=vap(eft_b[:], 0, [[KB * 128, 32], [1, kb * 128]]),
                    in_=vap(eft_d.ap(), b * KB * 128,
                            [[E_PAD, 32], [1, kb * 128]]))
                oh_b = bpool.tile([128, KB * 128], ohdt, tag="ohb")
                nc.sync.dma_start(
                    out=vap(oh_b[:], 0, [[KB * 128, 128], [1, kb * 128]]),
                    in_=vap(oh_d.ap(), b * 128 * KB * 128,
                            [[KB * 128, 128], [1, kb * 128]]))
                return basis_b, fu_b, eft_b, oh_b

            def mlp_bucket(b, eft_b, kb):
                rwb = rpool.tile([128, KB * 768], cdt, tag="rwb")
                for i in range(kb):
                    hps = hpool.tile([64, 128], f32, tag="hps")
                    nc.tensor.matmul(out=hps[:], lhsT=w1t_t[:],
                                     rhs=eft_b[:, i * 128:(i + 1) * 128],
                                     start=True, stop=True)
                    h65 = h65s[i % 2]
                    nc.scalar.activation(h65[0:64, :], hps[:], Act.Relu,
                                         bias=b1_t[:, 0:1])
                    rwp = ppool.tile([128, 768], f32, tag="rwp")
                    nc.tensor.matmul(out=rwp[:, 0:512], lhsT=h65[:],
                                     rhs=w2b_t[:, 0:512], start=True,
                                     stop=True)
                    nc.tensor.matmul(out=rwp[:, 512:768], lhsT=h65[:],
                                     rhs=w2b_t[:, 512:768], start=True,
                                     stop=True)
                    nc.scalar.activation(rwb[:, i * 768:(i + 1) * 768],
                                         rwp[:], Act.Copy)
                return rwb

            def tmp_bucket(b, basis_b, fu_b, kb):
                """fU x basis products + d1-sum -> tmp2 in (d2, j) layout.

                The d1-sum is a 2-level halving tree instead of a
                tensor_reduce: reduce only has a 1x uop (FD cycles), while
                the first tree level runs at 2x (pairs (d1,d1+2) keep both
                operands stride-1 / 4B-aligned). L2 pairs are stride-2 ->
                1x, but its FD is only a quarter of the reduce's.
                """
                tmp2 = gpool.tile([128, KB * 128], cdt, tag="tmp2")
                tmp2a = tmp2[:]
                ptb = gpool.tile([128, KB * 512], tdt, tag="ptb")
                ptba = ptb[:]
                # ptb layout per edge: [pd(16):32][m(8):4][d1(4):1]; the host
                # permutes basis cols to pd=(d2,p) and w2 rows to j=(p,m), so
                # the tree's natural group order 8*pd+m lands tmp2 directly
                # in P0's [d2:32][(p,m):1] layout -- every tree op then has a
                # unit-stride output (strided DVE writes cost ~4.3 cyc/elem).
                nc.vector.tensor_tensor(
                    vap(ptba, 0,
                        [[KB * 512, 128], [512, kb], [32, 16], [4, 8],
                         [1, 4]]),
                    vap(fu_b[:], 0,
                        [[KB * 32, 128], [32, kb], [0, 16], [4, 8],
                         [1, 4]]),
                    vap(basis_b[:], 0,
                        [[KB * 64, 128], [64, kb], [4, 16], [0, 8],
                         [1, 4]]),
                    Alu.mult)
                h1 = tpool.tile([128, KB * 256], cdt, tag="h1")
                h1a = h1[:]
                with nc.allow_low_precision(reason="fp16 tmp"):
                    nc.vector.tensor_tensor(
                        vap(h1a, 0,
                            [[KB * 256, 128], [256, kb], [2, 128], [1, 2]]),
                        vap(ptba, 0,
                            [[KB * 512, 128], [512, kb], [4, 128], [1, 2]]),
                        vap(ptba, 2,
                            [[KB * 512, 128], [512, kb], [4, 128], [1, 2]]),
                        Alu.add)
                    nc.vector.tensor_tensor(
                        vap(tmp2a, 0,
                            [[KB * 128, 128], [128, kb], [1, 128]]),
                        vap(h1a, 0,
                            [[KB * 256, 128], [256, kb], [2, 128]]),
                        vap(h1a, 1,
                            [[KB * 256, 128], [256, kb], [2, 128]]),
                        Alu.add)
                return tmp2

            # Pair-level tiles: the cascade fills half-slots per bucket and
            # the score/payload ops run once per 2 buckets with doubled FD,
            # halving their fixed per-instruction cost (~151 DVE cycles).
            KB2 = 2 * KB
            conv2 = tpool.tile([128, KB2 * 64], cdt, name="conv2")
            convV2 = tpool.tile([128, KB2 * 32], bf16, name="convV2")
            Y2 = tpool.tile([128, KB2 * 72], bf16, name="Y2")
            ex82 = tpool.tile([128, KB2 * 64], bf16, name="ex82")
            ps2 = tpool.tile([128, KB2 * 32], cdt, name="ps2")
            sc42 = tpool.tile([128, KB2 * 4], f32, name="sc42")
            scl2 = tpool.tile([128, KB2 * 4], f32, name="scl2")

            def conv_bucket(b, rwb, tmp2, kb):
                rwba = rwb[:]
                tmp2a = tmp2[:]
                half = (b & 1) * kb
                P0 = tpool.tile([128, KB * 3072], cdt, tag="P0")
                T1 = tpool.tile([128, KB * 1536], cdt, tag="T1")
                T2 = tpool.tile([128, KB * 768], cdt, tag="T2")
                T3 = tpool.tile([128, KB * 384], cdt, tag="T3")
                T4 = tpool.tile([128, KB * 192], cdt, tag="T4")
                nc.vector.tensor_tensor(
                    vap(P0[:], 0,
                        [[KB * 3072, 128], [3072, kb], [128, 24],
                         [32, 4], [1, 32]]),
                    vap(rwba, 0,
                        [[KB * 768, 128], [768, kb], [32, 24], [0, 4],
                         [1, 32]]),
                    vap(tmp2a, 0,
                        [[KB * 128, 128], [128, kb], [0, 24], [32, 4],
                         [1, 32]]),
                    Alu.mult)
                cur, w = P0[:], 32
                nxts = {16: T1, 8: T2, 4: T3, 2: T4}
                while w > 2:
                    w2 = w // 2
                    nxt = nxts[w2]
                    nc.vector.tensor_tensor(
                        vap(nxt[:], 0,
                            [[KB * 96 * w2, 128], [96 * w2, kb],
                             [4 * w2, 24], [w2, 4], [1, w2]]),
                        vap(cur, 0,
                            [[KB * 96 * w, 128], [96 * w, kb],
                             [4 * w, 24], [w, 4], [1, w2]]),
                        vap(cur, w2,
                            [[KB * 96 * w, 128], [96 * w, kb],
                             [4 * w, 24], [w, 4], [1, w2]]),
                        Alu.add)
                    cur, w = nxt[:], w2
                nc.vector.tensor_tensor(
                    vap(conv2[:], half * 64,
                        [[KB2 * 64, 128], [64, kb], [4, 16], [1, 4]]),
                    vap(cur, 0,
                        [[KB * 192, 128], [192, kb], [8, 16], [2, 4]]),
                    vap(cur, 1,
                        [[KB * 192, 128], [192, kb], [8, 16], [2, 4]]),
                    Alu.add)
                nc.vector.tensor_tensor(
                    vap(convV2[:], half * 32,
                        [[KB2 * 32, 128], [32, kb], [4, 8], [1, 4]]),
                    vap(cur, 128,
                        [[KB * 192, 128], [192, kb], [8, 8], [2, 4]]),
                    vap(cur, 129,
                        [[KB * 192, 128], [192, kb], [8, 8], [2, 4]]),
                    Alu.add)

            def scores_pair(b0, oh_pair, kb):
                """Scores + dual-exp softmax + payload for buckets b0,b0+1."""
                kb2 = 2 * kb
                Ya = Y2[:]
                nc.vector.tensor_tensor(
                    vap(ps2[:], 0,
                        [[KB2 * 32, 128], [1, kb2 * 32]]),
                    vap(conv2[:], 0,
                        [[KB2 * 64, 128], [64, kb2], [1, 32]]),
                    vap(conv2[:], 32,
                        [[KB2 * 64, 128], [64, kb2], [1, 32]]),
                    Alu.mult)
                nc.vector.tensor_reduce(
                    vap(sc42[:], 0,
                        [[KB2 * 4, 128], [1, kb2 * 4]]),
                    vap(ps2[:], 0,
                        [[KB2 * 32, 128], [8, kb2 * 4], [1, 8]]),
                    mybir.AxisListType.X, Alu.add)
                # SCALE is folded into the k-rows of w2 on the host.
                # (ScalarE Lrelu was tried here: it thrashes the activation
                # table -- 1.3us ACT_TABLE_LOAD per swap vs Exp -- and its
                # alpha lowering looked wrong on HW. Keep leaky on DVE.)
                sc4a = vap(sc42[:], 0, [[KB2 * 4, 128], [1, kb2 * 4]])
                scla = vap(scl2[:], 0, [[KB2 * 4, 128], [1, kb2 * 4]])
                nc.vector.scalar_tensor_tensor(
                    scla, sc4a, 0.2, sc4a, Alu.mult, Alu.max)
                nc.scalar.activation(
                    vap(Ya, 32, [[KB2 * 72, 128], [72, kb2], [1, 4]]),
                    scla, Act.Exp)
                nc.vector.tensor_scalar(
                    vap(Ya, 32, [[KB2 * 72, 128], [72, kb2], [1, 4]]),
                    vap(Ya, 32, [[KB2 * 72, 128], [72, kb2], [1, 4]]),
                    CLAMP_A, None, Alu.min)
                nc.scalar.activation(
                    vap(Ya, 68, [[KB2 * 72, 128], [72, kb2], [1, 4]]),
                    scla, Act.Exp, bias=shiftB[:, 0:1])
                # Pre-expand the 4 per-head exp values to 8-wide on the
                # ScalarE so the payload multiply keeps stride-1 operands
                # (2x DVE mode); a [0,8]-broadcast operand would force 1x.
                nc.scalar.activation(
                    vap(ex82[:], 0,
                        [[KB2 * 64, 128], [64, kb2], [32, 2], [8, 4],
                         [1, 8]]),
                    vap(Ya, 32,
                        [[KB2 * 72, 128], [72, kb2], [36, 2], [1, 4],
                         [0, 8]]),
                    Act.Copy)
                nc.vector.tensor_tensor(
                    vap(Ya, 0,
                        [[KB2 * 72, 128], [72, kb2], [36, 2], [8, 4],
                         [1, 8]]),
                    vap(convV2[:], 0,
                        [[KB2 * 32, 128], [32, kb2], [0, 2], [8, 4],
                         [1, 8]]),
                    vap(ex82[:], 0,
                        [[KB2 * 64, 128], [64, kb2], [32, 2], [8, 4],
                         [1, 8]]),
                    Alu.mult)

                # ---- segment matmuls (one-hot from host), per bucket
                for h in range(2):
                    seg = spool.tile([128, 72], f32, tag="seg")
                    for i in range(kb):
                        nc.tensor.matmul(
                            out=seg[:],
                            lhsT=oh_pair[h][:, i * 128:(i + 1) * 128],
                            rhs=Y2[:, (h * kb + i) * 72:
                                   (h * kb + i + 1) * 72],
                            start=(i == 0), stop=(i == kb - 1))
                    nc.scalar.activation(
                        segS[:, (b0 + h) * 72:(b0 + h + 1) * 72], seg[:],
                        Act.Copy)

            # ---- software pipeline: MLP + tmp run one bucket ahead
            KBP = cfg.kbp if cfg.kbp is not None else (KB,) * (NBk // 2)
            prev = None
            oh_even = None
            for b in range(NBk):
                kb = KBP[b // 2]
                basis_b, fu_b, eft_b, oh_b = load_bucket(b, kb)
                rwb = mlp_bucket(b, eft_b, kb)
                tmp2 = tmp_bucket(b, basis_b, fu_b, kb)
                if prev is not None:
                    pb, prwb, ptmp2, poh, pkb = prev
                    conv_bucket(pb, prwb, ptmp2, pkb)
                    if pb & 1:
                        scores_pair(pb - 1, (oh_even, poh), pkb)
                    else:
                        oh_even = poh
                prev = (b, rwb, tmp2, oh_b, kb)
            pb, prwb, ptmp2, poh, pkb = prev
            conv_bucket(pb, prwb, ptmp2, pkb)
            scores_pair(pb - 1, (oh_even, poh), pkb)

            # ======== Phase 3: select pass, divide, project, store ========
            segA = segS[:]
            rdA = cpool.tile([128, NBk * 4], f32)
            nc.vector.tensor_scalar(
                rdA[:], vap(segA, 32, [[NBk * 72, 128], [72, NBk], [1, 4]]),
                1e-30, None, Alu.add)
            nc.vector.reciprocal(rdA[:], rdA[:])
            rdB = cpool.tile([128, NBk * 4], f32)
            nc.vector.tensor_scalar(
                rdB[:], vap(segA, 68, [[NBk * 72, 128], [72, NBk], [1, 4]]),
                1e-30, None, Alu.add)
            nc.vector.reciprocal(rdB[:], rdB[:])
            msk = cpool.tile([128, NBk * 4], f32)
            nc.vector.tensor_scalar(
                msk[:], vap(segA, 32, [[NBk * 72, 128], [72, NBk], [1, 4]]),
                SEL_TH, None, Alu.is_lt)
            oA = cpool.tile([128, NBk * 32], f32)
            nc.vector.tensor_tensor(
                vap(oA[:], 0, [[NBk * 32, 128], [32, NBk], [8, 4], [1, 8]]),
                vap(segA, 0, [[NBk * 72, 128], [72, NBk], [8, 4], [1, 8]]),
                vap(rdA[:], 0, [[NBk * 4, 128], [4, NBk], [1, 4], [0, 8]]),
                Alu.mult)
            oB = cpool.tile([128, NBk * 32], f32)
            nc.vector.tensor_tensor(
                vap(oB[:], 0, [[NBk * 32, 128], [32, NBk], [8, 4], [1, 8]]),
                vap(segA, 36, [[NBk * 72, 128], [72, NBk], [8, 4], [1, 8]]),
                vap(rdB[:], 0, [[NBk * 4, 128], [4, NBk], [1, 4], [0, 8]]),
                Alu.mult)
            osc = cpool.tile([128, NBk * 32], bf16)
            osca = osc[:]
            nc.vector.tensor_tensor(oA[:], oA[:], oB[:], Alu.subtract)
            nc.vector.tensor_tensor(
                vap(oA[:], 0, [[NBk * 32, 128], [32, NBk], [8, 4], [1, 8]]),
                vap(oA[:], 0, [[NBk * 32, 128], [32, NBk], [8, 4], [1, 8]]),
                vap(msk[:], 0, [[NBk * 4, 128], [4, NBk], [1, 4], [0, 8]]),
                Alu.mult)
            nc.vector.tensor_tensor(osc[:], oA[:], oB[:], Alu.add)
            res = cpool.tile([128, NBk * 32], bf16)
            resa = res[:]
            scr = cpool.tile([128, NBk * 32], bf16)
            scra = scr[:]
            for mp in range(8):
                tgt = resa if mp == 0 else scra
                nc.vector.tensor_tensor(
                    vap(tgt, 0, [[NBk * 32, 128], [32, NBk], [4, 8], [1, 4]]),
                    vap(osca, mp * 4,
                        [[NBk * 32, 128], [32, NBk], [0, 8], [1, 4]]),
                    vap(proj_t[:], mp * 32,
                        [[256, 128], [0, NBk], [4, 8], [1, 4]]),
                    Alu.mult)
                if mp > 0:
                    nc.vector.tensor_tensor(resa, resa, scra, Alu.add)
            nc.sync.dma_start(
                out=vap(out_d.ap(), 0, [[32, 128], [4096, NBk], [1, 32]]),
                in_=res[:])
    return nc


# ---------------------------------------------------------------------------
# Host-side prep
# ---------------------------------------------------------------------------
def _host_ln(features, ln_w, ln_b):
    f32 = np.float32
    feats = features.reshape(-1, M1, D1).astype(f32)
    onehot = np.eye(2, dtype=f32)[IX1]
    norms = np.sqrt((feats ** 2) @ onehot)
    x = norms.reshape(-1, 2, 8)
    mu = x.mean(-1, keepdims=True, dtype=f32).astype(f32)
    var = ((x - mu) ** 2).mean(-1, keepdims=True, dtype=f32).astype(f32)
    ln = (x - mu) / np.sqrt(var + LN_EPS) * ln_w + ln_b
    ln = np.maximum(ln, 0).astype(f32).reshape(-1, M1, 2)
    return (feats * (ln / (norms + EQ_EPS))[:, :, IX1]).astype(f32)


def _prep(inputs, cfg: Cfg = None):
    mdt = np.float32 if OPT["mlp_f32"] else np.float16
    tdt = np.float32 if OPT["tmp_f32"] else np.float16

    src = np.asarray(inputs["src"]).astype(np.int64)
    dst = np.asarray(inputs["dst"]).astype(np.int64)
    n_nodes = np.asarray(inputs["features"]).shape[0]
    # basis: (E, d1, pd) -> per-edge (pd, d1) so products are stride-1.
    # pd columns permuted from k=(p,d2) to pd_hat=(d2,p) so the device
    # tree's group order 8*pd_hat+m equals 32*d2 + (8p+m) = P0's layout.
    basis = np.asarray(inputs["basis"], np.float32).reshape(-1, 4, 16)
    pd_perm = np.array([p * 4 + d2 for d2 in range(4) for p in range(4)])
    basis = basis[:, :, pd_perm]
    basis = np.ascontiguousarray(basis.transpose(0, 2, 1)).reshape(-1, 64)
    ef = np.asarray(inputs["edge_feats"], np.float32)

    nb_l = cfg.nb if cfg is not None else NB
    nb_g = N_CORES * nb_l
    nodes_pad = nb_g * BUCKET_N

    deg = np.bincount(dst, minlength=nodes_pad)
    order = np.argsort(-deg, kind="stable")
    assign = np.empty(nodes_pad, dtype=np.int64)
    pos = np.empty(nodes_pad, dtype=np.int64)
    # Phase 1: global degree-snake, only to balance edges across CORES.
    for r in range(BUCKET_N):
        sl = order[r * nb_g:(r + 1) * nb_g]
        buckets = np.arange(nb_g) if r % 2 == 0 else np.arange(nb_g)[::-1]
        assign[sl] = buckets
        pos[sl] = r
    # Phase 2: per-core repack -- the last LOW_N buckets of each core are
    # packed light (<= 7*128 edges) so their pairs run with one fewer
    # 128-edge chunk; the rest stay <= 8*128.  A sliding window over the
    # degree-sorted nodes picks the light subset; each group is deficit-
    # snaked into its buckets, then cap overshoots are repaired by node
    # swaps.  LOW_N=6 is attempted first and validated; fall back to the
    # always-feasible LOW_N=4 if the caps cannot be met.
    LOW_CAP, HI_CAP = 7 * BUCKET_N, 8 * BUCKET_N

    def _pack_core(core, LOW_N):
        idx = np.where(core_of == core)[0]
        idx = idx[np.argsort(-deg[idx], kind="stable")]
        nlow = LOW_N * BUCKET_N
        csum = np.concatenate([[0], np.cumsum(deg[idx])])
        wsums = csum[nlow:] - csum[:-nlow]
        w0 = int(np.argmin(np.abs(wsums - LOW_N * (LOW_CAP - 6))))
        low_idx = idx[w0:w0 + nlow]
        hi_idx = np.concatenate([idx[:w0], idx[w0 + nlow:]])
        nhi = nb_l - LOW_N
        buckets = [[] for _ in range(nb_l)]
        for grp_idx, nb_grp, off in ((hi_idx, nhi, 0),
                                     (low_idx, LOW_N, nhi)):
            tgt = float(deg[grp_idx].sum()) / nb_grp
            lb = np.zeros(nb_grp)
            for r in range(BUCKET_N):
                grp = grp_idx[r * nb_grp:(r + 1) * nb_grp]
                bo = np.argsort(-(tgt - lb), kind="stable")
                for t, n_ in enumerate(grp):
                    buckets[off + bo[t]].append(n_)
                np.add.at(lb, bo, deg[grp].astype(np.float64))
        loads_c = np.array([sum(int(deg[n_]) for n_ in bk)
                            for bk in buckets])
        caps = np.array([HI_CAP] * nhi + [LOW_CAP] * LOW_N)
        for _ in range(2000):
            over = np.where(loads_c > caps)[0]
            if len(over) == 0:
                break
            o = int(over[0])
            excess = int(loads_c[o] - caps[o])
            u = int(np.argmin(loads_c - caps))
            da = deg[np.asarray(buckets[o])]
            db = deg[np.asarray(buckets[u])]
            diffs = da[:, None] - db[None, :]
            room = int(caps[u] - loads_c[u])
            valid = (diffs > 0) & (diffs <= room)
            if not valid.any():
                return None
            score = np.where(valid, np.abs(diffs - excess), 1 << 30)
            ai, bi = np.unravel_index(np.argmin(score), score.shape)
            a, b_ = buckets[o][ai], buckets[u][bi]
            buckets[o][ai], buckets[u][bi] = b_, a
            d = int(deg[a] - deg[b_])
            loads_c[o] -= d
            loads_c[u] += d
        if (loads_c > caps).any():
            return None
        return buckets

    core_of = assign // nb_l
    for core in range(N_CORES):
        packed = _pack_core(core, 6)
        if packed is None:
            packed = _pack_core(core, 4)
        assert packed is not None, "bucket packing infeasible"
        for lb_i, bk in enumerate(packed):
            bk = np.asarray(bk)
            assign[bk] = core * nb_l + lb_i
            pos[bk] = np.arange(len(bk))
    loads = np.zeros(nb_g, dtype=np.int64)
    np.add.at(loads, assign[dst], 1)
    kb = int(math.ceil(loads.max() / 128.0))
    kb += kb & 1                       # pair split needs even max kb
    # Per-pair chunk counts: max over the two buckets and all 8 cores.
    kb_bkt = np.ceil(loads.reshape(N_CORES, nb_l) / 128.0).astype(int)
    kb_bkt = kb_bkt.max(axis=0)        # SPMD: one NEFF for all cores
    kbp = tuple(int(max(kb_bkt[2 * p], kb_bkt[2 * p + 1]))
                for p in range(nb_l // 2))
    if cfg is None:
        cfg = Cfg(nb=nb_l, kb=kb, kbp=kbp)
    assert kb <= cfg.kb, f"kb={kb} exceeds cfg.kb={cfg.kb}"

    f = _host_ln(np.asarray(inputs["features"], np.float32),
                 np.asarray(inputs["ln_w"], np.float32),
                 np.asarray(inputs["ln_b"], np.float32))
    fU_all = f[src].reshape(-1, 32)

    eb = assign[dst]
    eorder = np.argsort(eb, kind="stable")
    bstart = np.searchsorted(eb[eorder], np.arange(nb_g + 1))

    E_PAD, CH, KB = cfg.e_pad, cfg.ch, cfg.kb
    arange128 = np.arange(128, dtype=np.int64)
    in_maps = []
    for core in range(N_CORES):
        basis_s = np.zeros((E_PAD, 64), np.float32)
        fu_s = np.zeros((E_PAD, 32), np.float32)
        eft_s = np.zeros((32, E_PAD), np.float32)
        dstrel_s = np.full((E_PAD,), -1, np.int64)
        for lb in range(cfg.nb):
            gb = core * cfg.nb + lb
            eidx = eorder[bstart[gb]:bstart[gb + 1]]
            n = len(eidx)
            assert n <= cfg.kbp[lb // 2] * 128 if cfg.kbp else n <= KB * 128
            o = lb * KB * 128
            basis_s[o:o + n] = basis[eidx]
            fu_s[o:o + n] = fU_all[eidx]
            eft_s[:, o:o + n] = ef[eidx].T
            dstrel_s[o:o + n] = pos[dst[eidx]]
        # bucket-block layouts: (NB, 128, KB*w); edge (chunk i, part p)
        basis_bb = (basis_s.reshape(cfg.nb, KB, 128, 64)
                    .transpose(0, 2, 1, 3).reshape(cfg.nb, 128, KB * 64))
        fu_bb = (fu_s.reshape(cfg.nb, KB, 128, 32)
                 .transpose(0, 2, 1, 3).reshape(cfg.nb, 128, KB * 32))
        # one-hot: (NB, KB, 128e) -> (NB, 128e, KB*128n)
        oh_np = (ml_dtypes.float8_e4m3 if OPT.get("oh_fp8") else BF16)
        oh = (dstrel_s.reshape(cfg.nb, KB, 128)[..., None] ==
              arange128).astype(oh_np)
        oh_bb = np.ascontiguousarray(
            oh.transpose(0, 2, 1, 3)).reshape(cfg.nb, 128, KB * 128)
        in_maps.append({
            "basis_s": np.ascontiguousarray(basis_bb).astype(tdt),
            "fu_s": np.ascontiguousarray(fu_bb).astype(tdt),
            "eft_s": eft_s.astype(mdt),
            "oh_s": oh_bb,
        })

    w1 = np.asarray(inputs["w1"], np.float32)
    b1 = np.asarray(inputs["b1"], np.float32).reshape(64, 1)
    w2 = np.asarray(inputs["w2"], np.float32)
    b2 = np.asarray(inputs["b2"], np.float32)
    w2s = w2.copy()
    b2s = b2.copy()
    w2s[0:256, :] *= SCALE          # k-rows: fold the attention scale in
    b2s[0:256] *= SCALE
    # rw column order (i, m, p) -> (i, p, m): matches tmp2's (p, m) inner
    # layout so P0's stride-1 inner dim pairs rw and tmp consistently.
    jp = np.array([p * 8 + m for m in range(8) for p in range(4)])
    rperm = (np.arange(24)[:, None] * 32 + jp[None, :]).ravel()
    inv = np.empty_like(rperm)
    inv[rperm] = np.arange(768)
    w2s = w2s[inv]
    b2s = b2s[inv]
    w2b = np.concatenate([w2s.T, b2s[None, :]], axis=0).astype(np.float32)
    projw = np.asarray(inputs["proj_w"], np.float32)
    ptbl_flat = np.zeros((256,), np.float32)
    for mpi in range(8):
        for m in range(8):
            for d in range(4):
                ptbl_flat[mpi * 32 + m * 4 + d] = projw[IX2[d] * 8 + m, mpi]
    ptbl = np.broadcast_to(ptbl_flat, (128, 256)).astype(BF16)
    for im in in_maps:
        im.update({
            "w1t_s": np.ascontiguousarray(w1.T).astype(mdt),
            "b1_s": b1,
            "w2b_s": w2b.astype(mdt),
            "proj_s": ptbl,
        })
    meta = {"assign": assign, "pos": pos, "n_nodes": n_nodes}
    return in_maps, meta, cfg


def _unshard(results, meta):
    out_cat = np.concatenate([r["out_s"] for r in results], axis=0)
    assign, pos, n = meta["assign"], meta["pos"], meta["n_nodes"]
    rows = assign[:n] * 128 + pos[:n]
    return out_cat[rows].reshape(n, M2, D2)


def _run(inputs, trace=False):
    _apply_patches()
    import concourse.bass as bass
    from concourse.bass_utils import run_bass_kernel_spmd

    in_maps, meta, cfg = _prep(inputs)
    nc = bass.Bass("TRN2", target_bir_lowering=False)
    build_kernel(nc, cfg)
    r = run_bass_kernel_spmd(nc, in_maps, core_ids=list(range(N_CORES)),
                             trace=trace)
    out = _unshard(r.results, meta)
    return out, r


def kernel(**inputs) -> np.ndarray:
    out, _ = _run(inputs, trace=False)
    return out.astype(np.float32)



# revision 32
# speedup vs baseline: 1.0229x; 1.0030x over previous
"""Trainium2 Bass kernel for nn_EquivariantTransformerBlock.

Strategy (8 NeuronCores, no collectives, no indirect DMA):
  - Host assigns each node to one of 320 buckets of 128 nodes (degree-
    balanced snake packing); every edge goes to the core owning its dst
    bucket, so all segment sums are core-local.
  - Host computes the (tiny) equivariant LayerNorm, gathers f[src], and
    precomputes per-chunk one-hot matrices (fp8) for the segment sums.
  - Device pipeline, bucket (KB x 128 edges) at a time, software-
    pipelined one bucket ahead (PE MLP + ScalarE casts + DMA run under
    the DVE cascade of the previous bucket):
      * edge MLP on the TensorE in fp16; PSUM->SBUF casts on ScalarE,
      * fU x basis and rw x tmp contractions as fp16 broadcast-product
        + halving-tree ops on the VectorE (fp16 keeps the DVE 2x mode
        with 4x finer rounding than bf16; scores need that precision),
      * attention scores with the 1/sqrt(h) scale folded into the
        k-rows of w2 on the host,
      * dual-exp softmax (shift-free: clamped exp(s) / exp(s-140) with
        per-node select in the epilogue) with bf16 payload range,
      * segment sums as one-hot matmuls accumulated in PSUM per bucket.

DVE scheduling rules this kernel is tuned around (hardware-measured):
  - tensor_tensor runs 2x (2 elem/cyc/lane) only with 16-bit dtypes and
    unit-stride, 4B-aligned innermost dims on ALL operands; ~151 cycles
    fixed cost per instruction.
  - tensor_reduce has only a 1x uop -> the d1-sum over the fU x basis
    products is a 2-level halving tree instead (L1 at 2x); the host
    permutes basis cols to pd=(d2,p) and w2 rows to j=(p,m) so every
    tree level writes unit-stride (strided writes cost ~4.3 cyc/elem).
  - the per-head exp values are pre-expanded 8-wide on the ScalarE so
    the payload multiply keeps 2x (a [0,8]-broadcast operand forces 1x).
  - score/softmax/payload ops run once per bucket PAIR (doubled FD,
    halved fixed cost); ScalarE Lrelu is avoided (ACT_TABLE_LOAD thrash
    + wrong alpha on HW).
  - mixed-kb packing: the last 3 bucket pairs of each core are packed
    light (<= 896 edges/bucket via a sliding-window node subset + cap-
    repair swaps) and run with kb=7 chunks instead of 8, trimming
    6/320 of the edge-padding work -- this hits the per-pair
    quantization floor (sum kb = ceil(max_core_edges/256)); all
    per-bucket APs/loops take kb per pair (SPMD: kbp is the max over
    cores, with graceful fallback to fewer light pairs if a core's
    packing is infeasible).
"""

import math
from contextlib import ExitStack
from dataclasses import dataclass

import numpy as np
import ml_dtypes

BF16 = ml_dtypes.bfloat16

N_NODES = 40000
N_EDGES = 320000
M1, D1 = 8, 4
M2, D2 = 8, 4
LN_EPS = 1e-5
EQ_EPS = 1e-8
IX1 = np.array([0, 1, 1, 1])
IX2 = np.array([0, 1, 1, 1])

N_CORES = 8
BUCKET_N = 128
NB = 40
SCALE = 32.0 ** -0.5
SHIFT_B = 140.0
CLAMP_A = 1e34
SEL_TH = 1e33

# Precision knobs (validated against numpy sim of the same pipeline):
#   mlp_f32: run the edge MLP matmuls in fp32
#   tmp_f32: run the fU x basis cascade in fp32 (quantize tmp2 to bf16)
OPT = {"mlp_f32": False, "tmp_f32": False, "tmp_gpsimd": False, "tmp_5d": False, "oh_fp8": True}


@dataclass
class Cfg:
    nb: int
    kb: int                      # max chunks per bucket (tile sizing)
    kbp: tuple = None            # per-pair chunk count (len nb//2)

    @property
    def ch(self):
        return self.nb * self.kb

    @property
    def e_pad(self):
        return self.ch * 128


# ---------------------------------------------------------------------------
# Patches: this walrus build allows at most ONE sync wait per instruction.
# ---------------------------------------------------------------------------
_PATCHED = False


def _apply_patches():
    global _PATCHED
    if _PATCHED:
        return
    _PATCHED = True
    import re as _re

    import orjson as _orjson

    import concourse.bass as _bass
    from concourse.tile import TileContext as _TC
    from concourse.vector_clock import ScopedClock as _SC, VectorClock as _VC

    def _drain_and_barrier(self, tick_clock, wait_clock):
        nc = self.nc
        gvals = [int(x) for x in _re.findall(r"\d+", repr(tick_clock.global_clock))]
        nz = [(p, v) for p, v in enumerate(gvals) if v > 0]
        if not nz:
            nc.sync.drain()
        for p, v in nz:
            pvc = _VC()
            pvc.require_at_least(p, v)
            d = nc.sync.drain()
            wait_clock.add_sem_waits(d.ins, _SC({None: pvc}))
        nc.all_engine_barrier()
        assert self.sems is not None
        popped = nc._tile_sem_poison_stack.pop()
        assert popped is self._sem_poison
        nc.clear_and_free_semaphores(list(self.sems.allocated().values()))
        nc.all_engine_barrier()

    def _split_multi_waits(data: bytes) -> bytes:
        j = _orjson.loads(data)
        for fn in j.get("functions", []):
            for bb in fn.get("blocks", []):
                out = []
                for ins in bb.get("instructions", []):
                    si = ins.get("sync_info")
                    ow = (si or {}).get("on_wait") or []
                    if len(ow) > 1:
                        for k, w in enumerate(ow[:-1]):
                            out.append({
                                "debug": ins.get("debug", 0),
                                "engine": ins["engine"],
                                "ins": [],
                                "name": f"{ins['name']}-spw{k}",
                                "opcode": "EventSemaphore",
                                "outs": [],
                                "sync_info": {"on_update": [], "on_wait": [w]},
                            })
                        si["on_wait"] = [ow[-1]]
                    out.append(ins)
                bb["instructions"] = out
        return _orjson.dumps(j)

    _orig_to_json_bytes = _bass.Bass.to_json_bytes

    def _to_json_bytes(self):
        return _split_multi_waits(_orig_to_json_bytes(self))

    _TC._drain_and_barrier = _drain_and_barrier
    _bass.Bass.to_json_bytes = _to_json_bytes


# ---------------------------------------------------------------------------
# Device kernel
# ---------------------------------------------------------------------------
def build_kernel(nc, cfg: Cfg):
    import concourse.bass as bass
    import concourse.mybir as mybir
    from concourse.tile import TileContext

    f32 = mybir.dt.float32
    bf16 = mybir.dt.bfloat16
    fp16 = mybir.dt.float16
    Alu = mybir.AluOpType
    Act = mybir.ActivationFunctionType

    NBk, KB, CH, E_PAD = cfg.nb, cfg.kb, cfg.ch, cfg.e_pad
    HB = KB // 2            # half-bucket chunk count for the big cascade
    mdt = f32 if OPT["mlp_f32"] else fp16
    tdt = f32 if OPT["tmp_f32"] else fp16
    cdt = fp16

    basis_d = nc.dram_tensor("basis_s", (NBk, 128, KB * 64), tdt,
                             kind="ExternalInput")
    fu_d = nc.dram_tensor("fu_s", (NBk, 128, KB * 32), tdt,
                          kind="ExternalInput")
    eft_d = nc.dram_tensor("eft_s", (32, E_PAD), mdt, kind="ExternalInput")
    fp8 = mybir.dt.float8e4
    ohdt = fp8 if OPT.get("oh_fp8") else bf16
    oh_d = nc.dram_tensor("oh_s", (NBk, 128, KB * 128), ohdt,
                          kind="ExternalInput")
    w1t_d = nc.dram_tensor("w1t_s", (32, 64), mdt, kind="ExternalInput")
    b1_d = nc.dram_tensor("b1_s", (64, 1), f32, kind="ExternalInput")
    w2b_d = nc.dram_tensor("w2b_s", (65, 768), mdt, kind="ExternalInput")
    proj_d = nc.dram_tensor("proj_s", (128, 256), bf16, kind="ExternalInput")
    out_d = nc.dram_tensor("out_s", (NBk * 128, 32), bf16,
                           kind="ExternalOutput")

    def vap(base, offset, dims):
        return bass.AP(base.tensor, base.offset + offset, dims)

    with TileContext(nc) as tc:
        with ExitStack() as ctx:
            cpool = ctx.enter_context(tc.tile_pool(name="consts", bufs=1))
            w1t_t = cpool.tile([32, 64], mdt)
            b1_t = cpool.tile([64, 1], f32)
            w2b_t = cpool.tile([65, 768], mdt)
            proj_t = cpool.tile([128, 256], bf16)
            segS = cpool.tile([128, NBk * 72], f32)
            shiftB = cpool.tile([128, 1], f32)
            nc.vector.memset(shiftB[:], -SHIFT_B)

            def load_consts():
                # issued AFTER the first bucket loads so the DVE's first
                # op (tmp product of bucket 0) is not queued behind them
                nc.sync.dma_start(out=w1t_t[:], in_=w1t_d.ap())
                nc.sync.dma_start(out=b1_t[:], in_=b1_d.ap())
                nc.sync.dma_start(out=w2b_t[:], in_=w2b_d.ap())
                nc.sync.dma_start(out=proj_t[:], in_=proj_d.ap())

            bpool = ctx.enter_context(tc.tile_pool(name="edges", bufs=4))
            rpool = ctx.enter_context(tc.tile_pool(name="rw", bufs=2))
            gpool = ctx.enter_context(tc.tile_pool(name="tmp", bufs=2))
            tpool = ctx.enter_context(tc.tile_pool(name="work", bufs=1))
            hpool = ctx.enter_context(
                tc.tile_pool(name="psH", bufs=2, space="PSUM"))
            ppool = ctx.enter_context(
                tc.tile_pool(name="psR", bufs=2, space="PSUM"))
            spool = ctx.enter_context(
                tc.tile_pool(name="psS", bufs=2, space="PSUM"))

            # persistent h65 pair with the ones-row set once
            h65s = [cpool.tile([65, 128], mdt, name=f"h65p{k}")
                    for k in range(2)]
            for t in h65s:
                nc.gpsimd.memset(t[64:65, :], 1.0)

            def load_bucket(b, kb):
                basis_b = bpool.tile([128, KB * 64], tdt, tag="basisb")
                nc.sync.dma_start(
                    out=vap(basis_b[:], 0, [[KB * 64, 128], [1, kb * 64]]),
                    in_=vap(basis_d.ap(), b * 128 * KB * 64,
                            [[KB * 64, 128], [1, kb * 64]]))
                fu_b = bpool.tile([128, KB * 32], tdt, tag="fub")
                nc.sync.dma_start(
                    out=vap(fu_b[:], 0, [[KB * 32, 128], [1, kb * 32]]),
                    in_=vap(fu_d.ap(), b * 128 * KB * 32,
                            [[KB * 32, 128], [1, kb * 32]]))
                eft_b = bpool.tile([32, KB * 128], mdt, tag="eftb")
                nc.sync.dma_start(
                    out=vap(eft_b[:], 0, [[KB * 128, 32], [1, kb * 128]]),
                    in_=vap(eft_d.ap(), b * KB * 128,
                            [[E_PAD, 32], [1, kb * 128]]))
                oh_b = bpool.tile([128, KB * 128], ohdt, tag="ohb")
                nc.sync.dma_start(
                    out=vap(oh_b[:], 0, [[KB * 128, 128], [1, kb * 128]]),
                    in_=vap(oh_d.ap(), b * 128 * KB * 128,
                            [[KB * 128, 128], [1, kb * 128]]))
                return basis_b, fu_b, eft_b, oh_b

            def mlp_bucket(b, eft_b, kb):
                rwb = rpool.tile([128, KB * 768], cdt, tag="rwb")
                for i in range(kb):
                    hps = hpool.tile([64, 128], f32, tag="hps")
                    nc.tensor.matmul(out=hps[:], lhsT=w1t_t[:],
                                     rhs=eft_b[:, i * 128:(i + 1) * 128],
                                     start=True, stop=True)
                    h65 = h65s[i % 2]
                    nc.scalar.activation(h65[0:64, :], hps[:], Act.Relu,
                                         bias=b1_t[:, 0:1])
                    rwp = ppool.tile([128, 768], f32, tag="rwp")
                    nc.tensor.matmul(out=rwp[:, 0:512], lhsT=h65[:],
                                     rhs=w2b_t[:, 0:512], start=True,
                                     stop=True)
                    nc.tensor.matmul(out=rwp[:, 512:768], lhsT=h65[:],
                                     rhs=w2b_t[:, 512:768], start=True,
                                     stop=True)
                    nc.scalar.activation(rwb[:, i * 768:(i + 1) * 768],
                                         rwp[:], Act.Copy)
                return rwb

            def tmp_bucket(b, basis_b, fu_b, kb):
                """fU x basis products + d1-sum -> tmp2 in (d2, j) layout.

                The d1-sum is a 2-level halving tree instead of a
                tensor_reduce: reduce only has a 1x uop (FD cycles), while
                the first tree level runs at 2x (pairs (d1,d1+2) keep both
                operands stride-1 / 4B-aligned). L2 pairs are stride-2 ->
                1x, but its FD is only a quarter of the reduce's.
                """
                tmp2 = gpool.tile([128, KB * 128], cdt, tag="tmp2")
                tmp2a = tmp2[:]
                ptb = gpool.tile([128, KB * 512], tdt, tag="ptb")
                ptba = ptb[:]
                # ptb layout per edge: [pd(16):32][m(8):4][d1(4):1]; the host
                # permutes basis cols to pd=(d2,p) and w2 rows to j=(p,m), so
                # the tree's natural group order 8*pd+m lands tmp2 directly
                # in P0's [d2:32][(p,m):1] layout -- every tree op then has a
                # unit-stride output (strided DVE writes cost ~4.3 cyc/elem).
                nc.vector.tensor_tensor(
                    vap(ptba, 0,
                        [[KB * 512, 128], [512, kb], [32, 16], [4, 8],
                         [1, 4]]),
                    vap(fu_b[:], 0,
                        [[KB * 32, 128], [32, kb], [0, 16], [4, 8],
                         [1, 4]]),
                    vap(basis_b[:], 0,
                        [[KB * 64, 128], [64, kb], [4, 16], [0, 8],
                         [1, 4]]),
                    Alu.mult)
                h1 = tpool.tile([128, KB * 256], cdt, tag="h1")
                h1a = h1[:]
                with nc.allow_low_precision(reason="fp16 tmp"):
                    nc.vector.tensor_tensor(
                        vap(h1a, 0,
                            [[KB * 256, 128], [256, kb], [2, 128], [1, 2]]),
                        vap(ptba, 0,
                            [[KB * 512, 128], [512, kb], [4, 128], [1, 2]]),
                        vap(ptba, 2,
                            [[KB * 512, 128], [512, kb], [4, 128], [1, 2]]),
                        Alu.add)
                    nc.vector.tensor_tensor(
                        vap(tmp2a, 0,
                            [[KB * 128, 128], [128, kb], [1, 128]]),
                        vap(h1a, 0,
                            [[KB * 256, 128], [256, kb], [2, 128]]),
                        vap(h1a, 1,
                            [[KB * 256, 128], [256, kb], [2, 128]]),
                        Alu.add)
                return tmp2

            # Pair-level tiles: the cascade fills half-slots per bucket and
            # the score/payload ops run once per 2 buckets with doubled FD,
            # halving their fixed per-instruction cost (~151 DVE cycles).
            KB2 = 2 * KB
            conv2 = tpool.tile([128, KB2 * 64], cdt, name="conv2")
            convV2 = tpool.tile([128, KB2 * 32], bf16, name="convV2")
            Y2 = tpool.tile([128, KB2 * 72], bf16, name="Y2")
            ex82 = tpool.tile([128, KB2 * 64], bf16, name="ex82")
            ps2 = tpool.tile([128, KB2 * 32], cdt, name="ps2")
            sc42 = tpool.tile([128, KB2 * 4], f32, name="sc42")
            scl2 = tpool.tile([128, KB2 * 4], f32, name="scl2")

            def conv_bucket(b, rwb, tmp2, kb):
                rwba = rwb[:]
                tmp2a = tmp2[:]
                half = (b & 1) * kb
                P0 = tpool.tile([128, KB * 3072], cdt, tag="P0")
                T1 = tpool.tile([128, KB * 1536], cdt, tag="T1")
                T2 = tpool.tile([128, KB * 768], cdt, tag="T2")
                T3 = tpool.tile([128, KB * 384], cdt, tag="T3")
                T4 = tpool.tile([128, KB * 192], cdt, tag="T4")
                nc.vector.tensor_tensor(
                    vap(P0[:], 0,
                        [[KB * 3072, 128], [3072, kb], [128, 24],
                         [32, 4], [1, 32]]),
                    vap(rwba, 0,
                        [[KB * 768, 128], [768, kb], [32, 24], [0, 4],
                         [1, 32]]),
                    vap(tmp2a, 0,
                        [[KB * 128, 128], [128, kb], [0, 24], [32, 4],
                         [1, 32]]),
                    Alu.mult)
                cur, w = P0[:], 32
                nxts = {16: T1, 8: T2, 4: T3, 2: T4}
                while w > 2:
                    w2 = w // 2
                    nxt = nxts[w2]
                    nc.vector.tensor_tensor(
                        vap(nxt[:], 0,
                            [[KB * 96 * w2, 128], [96 * w2, kb],
                             [4 * w2, 24], [w2, 4], [1, w2]]),
                        vap(cur, 0,
                            [[KB * 96 * w, 128], [96 * w, kb],
                             [4 * w, 24], [w, 4], [1, w2]]),
                        vap(cur, w2,
                            [[KB * 96 * w, 128], [96 * w, kb],
                             [4 * w, 24], [w, 4], [1, w2]]),
                        Alu.add)
                    cur, w = nxt[:], w2
                nc.vector.tensor_tensor(
                    vap(conv2[:], half * 64,
                        [[KB2 * 64, 128], [64, kb], [4, 16], [1, 4]]),
                    vap(cur, 0,
                        [[KB * 192, 128], [192, kb], [8, 16], [2, 4]]),
                    vap(cur, 1,
                        [[KB * 192, 128], [192, kb], [8, 16], [2, 4]]),
                    Alu.add)
                nc.vector.tensor_tensor(
                    vap(convV2[:], half * 32,
                        [[KB2 * 32, 128], [32, kb], [4, 8], [1, 4]]),
                    vap(cur, 128,
                        [[KB * 192, 128], [192, kb], [8, 8], [2, 4]]),
                    vap(cur, 129,
                        [[KB * 192, 128], [192, kb], [8, 8], [2, 4]]),
                    Alu.add)

            def scores_pair(b0, oh_pair, kb):
                """Scores + dual-exp softmax + payload for buckets b0,b0+1."""
                kb2 = 2 * kb
                Ya = Y2[:]
                nc.vector.tensor_tensor(
                    vap(ps2[:], 0,
                        [[KB2 * 32, 128], [1, kb2 * 32]]),
                    vap(conv2[:], 0,
                        [[KB2 * 64, 128], [64, kb2], [1, 32]]),
                    vap(conv2[:], 32,
                        [[KB2 * 64, 128], [64, kb2], [1, 32]]),
                    Alu.mult)
                nc.vector.tensor_reduce(
                    vap(sc42[:], 0,
                        [[KB2 * 4, 128], [1, kb2 * 4]]),
                    vap(ps2[:], 0,
                        [[KB2 * 32, 128], [8, kb2 * 4], [1, 8]]),
                    mybir.AxisListType.X, Alu.add)
                # SCALE is folded into the k-rows of w2 on the host.
                # (ScalarE Lrelu was tried here: it thrashes the activation
                # table -- 1.3us ACT_TABLE_LOAD per swap vs Exp -- and its
                # alpha lowering looked wrong on HW. Keep leaky on DVE.)
                sc4a = vap(sc42[:], 0, [[KB2 * 4, 128], [1, kb2 * 4]])
                scla = vap(scl2[:], 0, [[KB2 * 4, 128], [1, kb2 * 4]])
                nc.vector.scalar_tensor_tensor(
                    scla, sc4a, 0.2, sc4a, Alu.mult, Alu.max)
                nc.scalar.activation(
                    vap(Ya, 32, [[KB2 * 72, 128], [72, kb2], [1, 4]]),
                    scla, Act.Exp)
                nc.vector.tensor_scalar(
                    vap(Ya, 32, [[KB2 * 72, 128], [72, kb2], [1, 4]]),
                    vap(Ya, 32, [[KB2 * 72, 128], [72, kb2], [1, 4]]),
                    CLAMP_A, None, Alu.min)
                nc.scalar.activation(
                    vap(Ya, 68, [[KB2 * 72, 128], [72, kb2], [1, 4]]),
                    scla, Act.Exp, bias=shiftB[:, 0:1])
                # Pre-expand the 4 per-head exp values to 8-wide on the
                # ScalarE so the payload multiply keeps stride-1 operands
                # (2x DVE mode); a [0,8]-broadcast operand would force 1x.
                nc.scalar.activation(
                    vap(ex82[:], 0,
                        [[KB2 * 64, 128], [64, kb2], [32, 2], [8, 4],
                         [1, 8]]),
                    vap(Ya, 32,
                        [[KB2 * 72, 128], [72, kb2], [36, 2], [1, 4],
                         [0, 8]]),
                    Act.Copy)
                nc.vector.tensor_tensor(
                    vap(Ya, 0,
                        [[KB2 * 72, 128], [72, kb2], [36, 2], [8, 4],
                         [1, 8]]),
                    vap(convV2[:], 0,
                        [[KB2 * 32, 128], [32, kb2], [0, 2], [8, 4],
                         [1, 8]]),
                    vap(ex82[:], 0,
                        [[KB2 * 64, 128], [64, kb2], [32, 2], [8, 4],
                         [1, 8]]),
                    Alu.mult)

                # ---- segment matmuls (one-hot from host), per bucket
                for h in range(2):
                    seg = spool.tile([128, 72], f32, tag="seg")
                    for i in range(kb):
                        nc.tensor.matmul(
                            out=seg[:],
                            lhsT=oh_pair[h][:, i * 128:(i + 1) * 128],
                            rhs=Y2[:, (h * kb + i) * 72:
                                   (h * kb + i + 1) * 72],
                            start=(i == 0), stop=(i == kb - 1))
                    nc.scalar.activation(
                        segS[:, (b0 + h) * 72:(b0 + h + 1) * 72], seg[:],
                        Act.Copy)

            # ---- software pipeline: MLP + tmp run one bucket ahead;
            # bucket loads are issued one further ahead (bufs=4) so the
            # DMA queue stays primed and the early DVE gaps close.
            KBP = cfg.kbp if cfg.kbp is not None else (KB,) * (NBk // 2)
            prev = None
            oh_even = None
            pending = load_bucket(0, KBP[0])
            load_consts()
            for b in range(NBk):
                kb = KBP[b // 2]
                basis_b, fu_b, eft_b, oh_b = pending
                if b + 1 < NBk:
                    pending = load_bucket(b + 1, KBP[(b + 1) // 2])
                rwb = mlp_bucket(b, eft_b, kb)
                tmp2 = tmp_bucket(b, basis_b, fu_b, kb)
                if prev is not None:
                    pb, prwb, ptmp2, poh, pkb = prev
                    conv_bucket(pb, prwb, ptmp2, pkb)
                    if pb & 1:
                        scores_pair(pb - 1, (oh_even, poh), pkb)
                    else:
                        oh_even = poh
                prev = (b, rwb, tmp2, oh_b, kb)
            pb, prwb, ptmp2, poh, pkb = prev
            conv_bucket(pb, prwb, ptmp2, pkb)
            scores_pair(pb - 1, (oh_even, poh), pkb)

            # ======== Phase 3: select pass, divide, project, store ========
            segA = segS[:]
            rdA = cpool.tile([128, NBk * 4], f32)
            nc.vector.tensor_scalar(
                rdA[:], vap(segA, 32, [[NBk * 72, 128], [72, NBk], [1, 4]]),
                1e-30, None, Alu.add)
            nc.vector.reciprocal(rdA[:], rdA[:])
            rdB = cpool.tile([128, NBk * 4], f32)
            nc.vector.tensor_scalar(
                rdB[:], vap(segA, 68, [[NBk * 72, 128], [72, NBk], [1, 4]]),
                1e-30, None, Alu.add)
            nc.vector.reciprocal(rdB[:], rdB[:])
            msk = cpool.tile([128, NBk * 4], f32)
            nc.vector.tensor_scalar(
                msk[:], vap(segA, 32, [[NBk * 72, 128], [72, NBk], [1, 4]]),
                SEL_TH, None, Alu.is_lt)
            oA = cpool.tile([128, NBk * 32], f32)
            nc.vector.tensor_tensor(
                vap(oA[:], 0, [[NBk * 32, 128], [32, NBk], [8, 4], [1, 8]]),
                vap(segA, 0, [[NBk * 72, 128], [72, NBk], [8, 4], [1, 8]]),
                vap(rdA[:], 0, [[NBk * 4, 128], [4, NBk], [1, 4], [0, 8]]),
                Alu.mult)
            oB = cpool.tile([128, NBk * 32], f32)
            nc.vector.tensor_tensor(
                vap(oB[:], 0, [[NBk * 32, 128], [32, NBk], [8, 4], [1, 8]]),
                vap(segA, 36, [[NBk * 72, 128], [72, NBk], [8, 4], [1, 8]]),
                vap(rdB[:], 0, [[NBk * 4, 128], [4, NBk], [1, 4], [0, 8]]),
                Alu.mult)
            osc = cpool.tile([128, NBk * 32], bf16)
            osca = osc[:]
            nc.vector.tensor_tensor(oA[:], oA[:], oB[:], Alu.subtract)
            nc.vector.tensor_tensor(
                vap(oA[:], 0, [[NBk * 32, 128], [32, NBk], [8, 4], [1, 8]]),
                vap(oA[:], 0, [[NBk * 32, 128], [32, NBk], [8, 4], [1, 8]]),
                vap(msk[:], 0, [[NBk * 4, 128], [4, NBk], [1, 4], [0, 8]]),
                Alu.mult)
            nc.vector.tensor_tensor(osc[:], oA[:], oB[:], Alu.add)
            res = cpool.tile([128, NBk * 32], bf16)
            resa = res[:]
            scr = cpool.tile([128, NBk * 32], bf16)
            scra = scr[:]
            for mp in range(8):
                tgt = resa if mp == 0 else scra
                nc.vector.tensor_tensor(
                    vap(tgt, 0, [[NBk * 32, 128], [32, NBk], [4, 8], [1, 4]]),
                    vap(osca, mp * 4,
                        [[NBk * 32, 128], [32, NBk], [0, 8], [1, 4]]),
                    vap(proj_t[:], mp * 32,
                        [[256, 128], [0, NBk], [4, 8], [1, 4]]),
                    Alu.mult)
                if mp > 0:
                    nc.vector.tensor_tensor(resa, resa, scra, Alu.add)
            nc.sync.dma_start(
                out=vap(out_d.ap(), 0, [[32, 128], [4096, NBk], [1, 32]]),
                in_=res[:])
    return nc


# ---------------------------------------------------------------------------
# Host-side prep
# ---------------------------------------------------------------------------
def _host_ln(features, ln_w, ln_b):
    f32 = np.float32
    feats = features.reshape(-1, M1, D1).astype(f32)
    onehot = np.eye(2, dtype=f32)[IX1]
    norms = np.sqrt((feats ** 2) @ onehot)
    x = norms.reshape(-1, 2, 8)
    mu = x.mean(-1, keepdims=True, dtype=f32).astype(f32)
    var = ((x - mu) ** 2).mean(-1, keepdims=True, dtype=f32).astype(f32)
    ln = (x - mu) / np.sqrt(var + LN_EPS) * ln_w + ln_b
    ln = np.maximum(ln, 0).astype(f32).reshape(-1, M1, 2)
    return (feats * (ln / (norms + EQ_EPS))[:, :, IX1]).astype(f32)


def _prep(inputs, cfg: Cfg = None):
    mdt = np.float32 if OPT["mlp_f32"] else np.float16
    tdt = np.float32 if OPT["tmp_f32"] else np.float16

    src = np.asarray(inputs["src"]).astype(np.int64)
    dst = np.asarray(inputs["dst"]).astype(np.int64)
    n_nodes = np.asarray(inputs["features"]).shape[0]
    # basis: (E, d1, pd) -> per-edge (pd, d1) so products are stride-1.
    # pd columns permuted from k=(p,d2) to pd_hat=(d2,p) so the device
    # tree's group order 8*pd_hat+m equals 32*d2 + (8p+m) = P0's layout.
    basis = np.asarray(inputs["basis"], np.float32).reshape(-1, 4, 16)
    pd_perm = np.array([p * 4 + d2 for d2 in range(4) for p in range(4)])
    basis = basis[:, :, pd_perm]
    basis = np.ascontiguousarray(basis.transpose(0, 2, 1)).reshape(-1, 64)
    ef = np.asarray(inputs["edge_feats"], np.float32)

    nb_l = cfg.nb if cfg is not None else NB
    nb_g = N_CORES * nb_l
    nodes_pad = nb_g * BUCKET_N

    deg = np.bincount(dst, minlength=nodes_pad)
    order = np.argsort(-deg, kind="stable")
    assign = np.empty(nodes_pad, dtype=np.int64)
    pos = np.empty(nodes_pad, dtype=np.int64)
    # Phase 1: global degree-snake, only to balance edges across CORES.
    for r in range(BUCKET_N):
        sl = order[r * nb_g:(r + 1) * nb_g]
        buckets = np.arange(nb_g) if r % 2 == 0 else np.arange(nb_g)[::-1]
        assign[sl] = buckets
        pos[sl] = r
    # Phase 2: per-core repack -- the last LOW_N buckets of each core are
    # packed light (<= 7*128 edges) so their pairs run with one fewer
    # 128-edge chunk; the rest stay <= 8*128.  A sliding window over the
    # degree-sorted nodes picks the light subset; each group is deficit-
    # snaked into its buckets, then cap overshoots are repaired by node
    # swaps.  LOW_N=6 is attempted first and validated; fall back to the
    # always-feasible LOW_N=4 if the caps cannot be met.
    LOW_CAP, HI_CAP = 7 * BUCKET_N, 8 * BUCKET_N

    def _pack_core(core, LOW_N):
        idx = np.where(core_of == core)[0]
        idx = idx[np.argsort(-deg[idx], kind="stable")]
        nlow = LOW_N * BUCKET_N
        csum = np.concatenate([[0], np.cumsum(deg[idx])])
        wsums = csum[nlow:] - csum[:-nlow]
        w0 = int(np.argmin(np.abs(wsums - LOW_N * (LOW_CAP - 6))))
        low_idx = idx[w0:w0 + nlow]
        hi_idx = np.concatenate([idx[:w0], idx[w0 + nlow:]])
        nhi = nb_l - LOW_N
        buckets = [[] for _ in range(nb_l)]
        for grp_idx, nb_grp, off in ((hi_idx, nhi, 0),
                                     (low_idx, LOW_N, nhi)):
            tgt = float(deg[grp_idx].sum()) / nb_grp
            lb = np.zeros(nb_grp)
            for r in range(BUCKET_N):
                grp = grp_idx[r * nb_grp:(r + 1) * nb_grp]
                bo = np.argsort(-(tgt - lb), kind="stable")
                for t, n_ in enumerate(grp):
                    buckets[off + bo[t]].append(n_)
                np.add.at(lb, bo, deg[grp].astype(np.float64))
        loads_c = np.array([sum(int(deg[n_]) for n_ in bk)
                            for bk in buckets])
        caps = np.array([HI_CAP] * nhi + [LOW_CAP] * LOW_N)
        for _ in range(2000):
            over = np.where(loads_c > caps)[0]
            if len(over) == 0:
                break
            o = int(over[0])
            excess = int(loads_c[o] - caps[o])
            u = int(np.argmin(loads_c - caps))
            da = deg[np.asarray(buckets[o])]
            db = deg[np.asarray(buckets[u])]
            diffs = da[:, None] - db[None, :]
            room = int(caps[u] - loads_c[u])
            valid = (diffs > 0) & (diffs <= room)
            if not valid.any():
                return None
            score = np.where(valid, np.abs(diffs - excess), 1 << 30)
            ai, bi = np.unravel_index(np.argmin(score), score.shape)
            a, b_ = buckets[o][ai], buckets[u][bi]
            buckets[o][ai], buckets[u][bi] = b_, a
            d = int(deg[a] - deg[b_])
            loads_c[o] -= d
            loads_c[u] += d
        if (loads_c > caps).any():
            return None
        return buckets

    core_of = assign // nb_l
    for core in range(N_CORES):
        packed = _pack_core(core, 6)
        if packed is None:
            packed = _pack_core(core, 4)
        assert packed is not None, "bucket packing infeasible"
        for lb_i, bk in enumerate(packed):
            bk = np.asarray(bk)
            assign[bk] = core * nb_l + lb_i
            pos[bk] = np.arange(len(bk))
    loads = np.zeros(nb_g, dtype=np.int64)
    np.add.at(loads, assign[dst], 1)
    kb = int(math.ceil(loads.max() / 128.0))
    kb += kb & 1                       # pair split needs even max kb
    # Per-pair chunk counts: max over the two buckets and all 8 cores.
    kb_bkt = np.ceil(loads.reshape(N_CORES, nb_l) / 128.0).astype(int)
    kb_bkt = kb_bkt.max(axis=0)        # SPMD: one NEFF for all cores
    kbp = tuple(int(max(kb_bkt[2 * p], kb_bkt[2 * p + 1]))
                for p in range(nb_l // 2))
    if cfg is None:
        cfg = Cfg(nb=nb_l, kb=kb, kbp=kbp)
    assert kb <= cfg.kb, f"kb={kb} exceeds cfg.kb={cfg.kb}"

    f = _host_ln(np.asarray(inputs["features"], np.float32),
                 np.asarray(inputs["ln_w"], np.float32),
                 np.asarray(inputs["ln_b"], np.float32))
    fU_all = f[src].reshape(-1, 32)

    eb = assign[dst]
    eorder = np.argsort(eb, kind="stable")
    bstart = np.searchsorted(eb[eorder], np.arange(nb_g + 1))

    E_PAD, CH, KB = cfg.e_pad, cfg.ch, cfg.kb
    arange128 = np.arange(128, dtype=np.int64)
    in_maps = []
    for core in range(N_CORES):
        basis_s = np.zeros((E_PAD, 64), np.float32)
        fu_s = np.zeros((E_PAD, 32), np.float32)
        eft_s = np.zeros((32, E_PAD), np.float32)
        dstrel_s = np.full((E_PAD,), -1, np.int64)
        for lb in range(cfg.nb):
            gb = core * cfg.nb + lb
            eidx = eorder[bstart[gb]:bstart[gb + 1]]
            n = len(eidx)
            assert n <= cfg.kbp[lb // 2] * 128 if cfg.kbp else n <= KB * 128
            o = lb * KB * 128
            basis_s[o:o + n] = basis[eidx]
            fu_s[o:o + n] = fU_all[eidx]
            eft_s[:, o:o + n] = ef[eidx].T
            dstrel_s[o:o + n] = pos[dst[eidx]]
        # bucket-block layouts: (NB, 128, KB*w); edge (chunk i, part p)
        basis_bb = (basis_s.reshape(cfg.nb, KB, 128, 64)
                    .transpose(0, 2, 1, 3).reshape(cfg.nb, 128, KB * 64))
        fu_bb = (fu_s.reshape(cfg.nb, KB, 128, 32)
                 .transpose(0, 2, 1, 3).reshape(cfg.nb, 128, KB * 32))
        # one-hot: (NB, KB, 128e) -> (NB, 128e, KB*128n)
        oh_np = (ml_dtypes.float8_e4m3 if OPT.get("oh_fp8") else BF16)
        oh = (dstrel_s.reshape(cfg.nb, KB, 128)[..., None] ==
              arange128).astype(oh_np)
        oh_bb = np.ascontiguousarray(
            oh.transpose(0, 2, 1, 3)).reshape(cfg.nb, 128, KB * 128)
        in_maps.append({
            "basis_s": np.ascontiguousarray(basis_bb).astype(tdt),
            "fu_s": np.ascontiguousarray(fu_bb).astype(tdt),
            "eft_s": eft_s.astype(mdt),
            "oh_s": oh_bb,
        })

    w1 = np.asarray(inputs["w1"], np.float32)
    b1 = np.asarray(inputs["b1"], np.float32).reshape(64, 1)
    w2 = np.asarray(inputs["w2"], np.float32)
    b2 = np.asarray(inputs["b2"], np.float32)
    w2s = w2.copy()
    b2s = b2.copy()
    w2s[0:256, :] *= SCALE          # k-rows: fold the attention scale in
    b2s[0:256] *= SCALE
    # rw column order (i, m, p) -> (i, p, m): matches tmp2's (p, m) inner
    # layout so P0's stride-1 inner dim pairs rw and tmp consistently.
    jp = np.array([p * 8 + m for m in range(8) for p in range(4)])
    rperm = (np.arange(24)[:, None] * 32 + jp[None, :]).ravel()
    inv = np.empty_like(rperm)
    inv[rperm] = np.arange(768)
    w2s = w2s[inv]
    b2s = b2s[inv]
    w2b = np.concatenate([w2s.T, b2s[None, :]], axis=0).astype(np.float32)
    projw = np.asarray(inputs["proj_w"], np.float32)
    ptbl_flat = np.zeros((256,), np.float32)
    for mpi in range(8):
        for m in range(8):
            for d in range(4):
                ptbl_flat[mpi * 32 + m * 4 + d] = projw[IX2[d] * 8 + m, mpi]
    ptbl = np.broadcast_to(ptbl_flat, (128, 256)).astype(BF16)
    for im in in_maps:
        im.update({
            "w1t_s": np.ascontiguousarray(w1.T).astype(mdt),
            "b1_s": b1,
            "w2b_s": w2b.astype(mdt),
            "proj_s": ptbl,
        })
    meta = {"assign": assign, "pos": pos, "n_nodes": n_nodes}
    return in_maps, meta, cfg


def _unshard(results, meta):
    out_cat = np.concatenate([r["out_s"] for r in results], axis=0)
    assign, pos, n = meta["assign"], meta["pos"], meta["n_nodes"]
    rows = assign[:n] * 128 + pos[:n]
    return out_cat[rows].reshape(n, M2, D2)


def _run(inputs, trace=False):
    _apply_patches()
    import concourse.bass as bass
    from concourse.bass_utils import run_bass_kernel_spmd

    in_maps, meta, cfg = _prep(inputs)
    nc = bass.Bass("TRN2", target_bir_lowering=False)
    build_kernel(nc, cfg)
    r = run_bass_kernel_spmd(nc, in_maps, core_ids=list(range(N_CORES)),
                             trace=trace)
    out = _unshard(r.results, meta)
    return out, r


def kernel(**inputs) -> np.ndarray:
    out, _ = _run(inputs, trace=False)
    return out.astype(np.float32)

